# revision 21
# baseline (speedup 1.0000x reference)
"""Trainium2 Bass kernel for nn_DistanceFusionBlock (retrieval_knn).

Sharding (8 NeuronCores, SPMD single NEFF): token-parallel — core c
handles batch b = c // 4, token quarter g = c % 4 (64 tokens) for BOTH
the v- and a-streams. Inputs arrive host-packed per core (transposed,
chunked, bf16) so no on-device transposes are needed.

Distance phase (the N^2*D part), using |x| = 2*relu(x) - x:
  - 256 gen tiles per core: t = relu(x_v[d,:] - x_a[d,j]) over all 256
    i (free dim), d-chunks on partitions, j in the core's own quarter.
    Split DVE tensor_scalar(sub,max0) [4x mode, 127ns] / ACT Relu with
    per-partition bias [398ns] at ACT_EVERY.
  - The PE folds every tile into row j of a [64,256] PSUM "rows" matrix
    via a sliding one-hot-column lhsT (matmul out base-partition must be
    0/32/64, so scattering is done with the weights, accumulating exact
    zeros elsewhere).
  - sum(diff) corrections are analytic from row/col sums of x_v / x_a
    (tiny PE folds): da_raw[j] = 2*rowsum_j - SV + 256*sa_j (local);
    dv partial = 2*colsum - 64*sv + SA, summed across the 4-core group
    by a 1KB ReduceScatter that also hands each core exactly its own
    64 tokens' slice.

MLP phase: features-on-partitions end-to-end; mm1 runs on RAW inputs
interleaved into the PE fold stream (row scaling commutes:
(dv*x) @ W = dv * (x @ W)); the dv/da scale is applied to the mm1
output (dv broadcast across partitions via a K=1 matmul that also
applies the 1/N), then gelu(+per-partition bias) on ACT, mm2, and the
concat-projection as one wide [128,4,64] PSUM accumulation over both
streams. bf16 operands, fp32 accumulation. The a-stream tail is fully
local and hides the ReduceScatter; only the v-stream tail is dv-gated.

Hardware constraint honored throughout: every TPB instruction has ONE
semaphore wait slot (see _split_multi_waits); per-engine absorber ops
retire each DMA-pack semaphore once so hot-loop ops carry at most one.
"""
import os
import sys

sys.path.insert(0, "/opt/trn_rl_repo")

import numpy as np
import ml_dtypes

import concourse.bass as bass
import concourse.mybir as mybir
import concourse.tile as tile
from concourse.bass import ds
from concourse.bass_utils import run_bass_kernel_spmd

B, N, D, H = 2, 256, 512, 2048
NCORES, GROUP, TOK = 8, 4, 64
DC, HC, OC = D // 128, H // 128, D // 128  # 4, 16, 4
BF, F32 = mybir.dt.bfloat16, mybir.dt.float32
ACT_EVERY = 4  # every ACT_EVERY-th gen tile goes to the scalar engine
SKIP_GEN = False
GEN_BUFS = 8
MM1_BASE_V = 72
MM1_BASE_A = 112
SKIP_MLP = False
SKIP_RS = False

# genpack_bf free-dim layout per d-chunk: [xvT(256) | xvO(64) | xaO(64)]
GBF_W = 384
# genpack_f32 layout per d-chunk: [xa_col(64) | -xa_col(64)]
GF_W = 128
# biaspack layout: [b1v(16) | b1a(16) | bmv(4) | bma(4) | bout(4)]
BIAS_W = 44
# weight pack layout (per stream): [W1(4*2048) | Wm(16*512) | Wout_half(4*512)]
WP_W1, WP_WM, WP_WO = 0, 4 * 2048, 4 * 2048 + 16 * 512
WP_W = WP_WO + 4 * 512  # 18432


def _split_multi_waits(nc):
    """Every TPB instruction struct has exactly ONE semaphore-wait slot;
    this snapshot's Tile doesn't split multi-wait instructions (its wait
    optimizer is disabled). Move all-but-one wait of any instruction onto
    injected same-engine NoOps placed immediately before it."""
    import bass_rust
    n = 0
    for fn in nc.m.functions:
        for blk in fn.blocks:
            out = []
            for ins in blk.instructions:
                si = ins.sync_info
                waits = list(si.on_wait) if si is not None and si.on_wait else []
                if len(waits) > 1:
                    for w in waits[:-1]:
                        nop = bass_rust.InstNoOp(
                            name=f"waitsplit-{n}", engine=ins.engine,
                            ins=[], outs=[])
                        nop.sync_info = mybir.SyncInfo(on_wait=[w], on_update=[])
                        out.append(nop)
                        n += 1
                    si.on_wait = [waits[-1]]
                out.append(ins)
            blk.instructions[:] = out
    return n


def build_bass():
    nc = bass.Bass(num_devices=NCORES)
    g_bf = nc.dram_tensor("g_bf", [128, DC * GBF_W], BF, kind="ExternalInput")
    g_f = nc.dram_tensor("g_f", [128, DC * GF_W + BIAS_W], F32, kind="ExternalInput")
    w_v = nc.dram_tensor("w_v", [128, WP_W], BF, kind="ExternalInput")
    w_a = nc.dram_tensor("w_a", [128, WP_W], BF, kind="ExternalInput")
    out_d = nc.dram_tensor("out", [OC, 128, TOK], F32, kind="ExternalOutput")

    with tile.TileContext(nc) as tc:
        with (
            tc.tile_pool(name="inp", bufs=1) as inp,
            tc.tile_pool(name="gen_d", bufs=GEN_BUFS) as genp_d,
            tc.tile_pool(name="diffp", bufs=3) as diffp,
            tc.tile_pool(name="gen_a", bufs=4) as genp_a,
            tc.tile_pool(name="sb", bufs=1) as sb,
            tc.tile_pool(name="ps_acc", bufs=1, space="PSUM") as ps_acc,
            tc.tile_pool(name="ps_misc", bufs=1, space="PSUM") as ps_misc,
            tc.tile_pool(name="ps_pe", bufs=4, space="PSUM") as ps_pe,
            tc.tile_pool(name="ps_dve", bufs=2, space="PSUM") as ps_dve,
            tc.tile_pool(name="dram", bufs=1, space="DRAM") as dram,
        ):
            # ---------------- input DMAs ----------------
            sb_gbf = inp.tile([128, DC * GBF_W], BF)
            sb_gf = inp.tile([128, DC * GF_W + BIAS_W], F32)
            sb_wv = inp.tile([128, WP_W], BF)
            sb_wa = inp.tile([128, WP_W], BF)
            nc.sync.dma_start(sb_gbf[:], g_bf[:])
            nc.sync.dma_start(sb_gf[:], g_f[:])
            if not SKIP_MLP:
                nc.sync.dma_start(sb_wv[:, ds(WP_W1, WP_WM)], w_v[:, ds(WP_W1, WP_WM)])
                nc.sync.dma_start(sb_wa[:, ds(WP_W1, WP_WM)], w_a[:, ds(WP_W1, WP_WM)])
                nc.sync.dma_start(sb_wv[:, ds(WP_WM, WP_W - WP_WM)], w_v[:, ds(WP_WM, WP_W - WP_WM)])
                nc.sync.dma_start(sb_wa[:, ds(WP_WM, WP_W - WP_WM)], w_a[:, ds(WP_WM, WP_W - WP_WM)])
            else:
                nc.sync.dma_start(sb_wv[:, 0:2], w_v[:, 0:2])
                nc.sync.dma_start(sb_wa[:, 0:2], w_a[:, 0:2])

            # ---------------- constants ----------------
            zeros = sb.tile([128, 256], BF)
            ones_bf = sb.tile([128, 1], BF)
            ones_f = sb.tile([128, 1], F32)
            c64_bf = sb.tile([128, 1], BF)
            scale_row = sb.tile([1, 128], F32)
            zo = sb.tile([128, 128], BF)  # single ones-column at index TOK
            ident = sb.tile([TOK, TOK], F32)
            nc.vector.memset(zeros[:], 0.0)
            nc.vector.memset(ones_bf[:], 1.0)
            nc.vector.memset(ones_f[:], 1.0)
            nc.vector.memset(c64_bf[:], float(TOK))
            nc.vector.memset(scale_row[:], 1.0 / N)
            nc.vector.memset(zo[:], 0.0)
            nc.vector.memset(zo[:, TOK:TOK + 1], 1.0)
            from concourse.masks import make_identity
            make_identity(nc, ident[:])

            # ---------------- per-engine semaphore absorbers ----------------
            # DVE: touch each DMA pack once (1 wait per op, dataflow-safe by
            # priority order).
            dve_scr = sb.tile([1, 2], F32)
            nc.vector.tensor_copy(dve_scr[0:1, 0:1], sb_gf[0:1, 0:1])
            dve_scr2 = sb.tile([1, 2], BF)
            nc.vector.tensor_copy(dve_scr2[0:1, 0:1], sb_gbf[0:1, 0:1])
            # ACT: same, plus warm the gelu/abs table set early.
            act_scr = sb.tile([1, 2], BF)
            nc.scalar.copy(act_scr[0:1, 0:1], sb_gbf[0:1, 0:1])
            act_scr2 = sb.tile([1, 2], F32)
            nc.scalar.copy(act_scr2[0:1, 0:1], sb_gf[0:1, 0:1])
            warm = sb.tile([128, 1], BF)
            nc.scalar.activation(warm[:], zeros[:, 0:1],
                                 mybir.ActivationFunctionType.Gelu)
            # PE: dummy 1-col matmuls absorbing each pack's semaphore.
            scr_ps = ps_misc.tile([1, 1], F32, tag="misc")
            nc.tensor.matmul(out=scr_ps[:], lhsT=ones_bf[:], rhs=ones_bf[:],
                             start=True, stop=True)
            scr_ps2 = ps_misc.tile([1, 1], F32, name="scr2", tag="misc")
            nc.tensor.matmul(out=scr_ps2[:], lhsT=ones_bf[:],
                             rhs=sb_gbf[:, 0:1], start=True, stop=True)

            # ---------------- distance phase ----------------
            # relu trick: |x| = 2*relu(x) - x, and sum(diff) is analytic.
            # Each tile t = relu(x_v[d,:] - x_a[d,j]); folds write row j of
            # rows_ps via a sliding one-hot column lhsT.
            # sv/sa ingredient folds first (their DVE tail overlaps gen)
            sv64_ps = ps_misc.tile([1, 256], F32, tag="misc")
            for dc in range(DC):
                nc.tensor.matmul(out=sv64_ps[:], lhsT=c64_bf[:],
                                 rhs=sb_gbf[:, ds(dc * GBF_W, 256)],
                                 start=(dc == 0), stop=(dc == DC - 1))
            sv64_sb = sb.tile([1, 256], F32)
            nc.vector.tensor_copy(sv64_sb[:], sv64_ps[:])
            sa_ps = ps_misc.tile([1, TOK], F32, tag="misc")
            for dc in range(DC):
                nc.tensor.matmul(out=sa_ps[:], lhsT=ones_bf[:],
                                 rhs=sb_gbf[:, ds(dc * GBF_W + 320, TOK)],
                                 start=(dc == 0), stop=(dc == DC - 1))
            sa_sb = sb.tile([1, TOK], F32)
            nc.vector.tensor_copy(sa_sb[:], sa_ps[:])
            sa_tot = sb.tile([1, 1], F32)
            nc.vector.tensor_reduce(sa_tot[:], sa_sb[:],
                                    axis=mybir.AxisListType.X,
                                    op=mybir.AluOpType.add)
            svq = sb.tile([1, 1], F32)
            nc.vector.tensor_reduce(svq[:], sv64_sb[:],
                                    axis=mybir.AxisListType.X,
                                    op=mybir.AluOpType.add)
            sv_tot = sb.tile([1, 1], F32)
            nc.vector.tensor_scalar(
                out=sv_tot[:], in0=svq[:], scalar1=1.0 / TOK, scalar2=None,
                op0=mybir.AluOpType.mult, op1=mybir.AluOpType.bypass)
            # mm1 raw matmuls are interleaved into the fold stream below:
            # by fold #F_V the W1v pack has landed, by #F_A W1a has.
            mm1_sched = {}
            z_ps = {}
            z_sb = {}
            if not SKIP_MLP:
                for s, base in (("v", MM1_BASE_V), ("a", MM1_BASE_A)):
                    z_sb[s] = sb.tile([128, HC, TOK], BF, name=f"z_{s}")
                    z_ps[s] = [None] * (HC // 4)
                    kk = 0
                    for hc in range(HC):
                        for dc in range(DC):
                            mm1_sched.setdefault(base + kk // 2, []).append((s, hc, dc))
                            kk += 1
            rows_ps = ps_acc.tile([TOK, 256], F32)
            njj = TOK if not SKIP_GEN else 1
            nfold = njj * DC
            k = 0
            for j in range(njj):
                for dc in range(DC):
                    use_act = k % ACT_EVERY == ACT_EVERY - 1
                    t = (genp_a if use_act else genp_d).tile(
                        [128, 256], BF, name="gt")
                    if use_act:
                        nc.scalar.activation(
                            t[:], sb_gbf[:, ds(dc * GBF_W, 256)],
                            mybir.ActivationFunctionType.Relu,
                            bias=sb_gf[:, ds(dc * GF_W + 64 + j, 1)],
                            scale=1.0,
                        )
                    else:
                        nc.vector.tensor_scalar(
                            out=t[:],
                            in0=sb_gbf[:, ds(dc * GBF_W, 256)],
                            scalar1=sb_gf[:, ds(dc * GF_W + j, 1)],
                            scalar2=0.0,
                            op0=mybir.AluOpType.subtract,
                            op1=mybir.AluOpType.max,
                        )
                    nc.tensor.matmul(
                        out=rows_ps[:], lhsT=zo[:, ds(TOK - j, TOK)],
                        rhs=t[:], start=(k == 0), stop=(k == nfold - 1))
                    for s, hc, dcw in mm1_sched.get(k, ()):
                        wp = sb_wv if s == "v" else sb_wa
                        grp = hc // 4
                        if z_ps[s][grp] is None:
                            z_ps[s][grp] = ps_pe.tile(
                                [128, 4, TOK], F32, name="zp", tag="pe")
                        nc.tensor.matmul(
                            out=z_ps[s][grp][:, hc % 4, :],
                            lhsT=wp[:, ds(WP_W1 + dcw * 2048 + hc * 128, 128)],
                            rhs=sb_gbf[:, ds(dcw * GBF_W + (256 if s == "v" else 320), TOK)],
                            start=(dcw == 0), stop=(dcw == DC - 1),
                        )
                        if dcw == DC - 1 and hc % 4 == 3:
                            nc.vector.tensor_copy(
                                z_sb[s][:, ds(grp * 4, 4), :], z_ps[s][grp][:])
                            z_ps[s][grp] = None
                    k += 1
            # rows -> SBUF
            rows_sb = sb.tile([TOK, 256], F32)
            nc.vector.tensor_copy(rows_sb[:], rows_ps[:])


            # ---------------- da (local, from rows + analytic corr) -------
            # da_raw[j] = 2*sum_i rows[j,i] - SV + 256*sa[j]
            rowsum = sb.tile([TOK, 1], F32)
            nc.vector.tensor_reduce(rowsum[:], rows_sb[:],
                                    axis=mybir.AxisListType.X,
                                    op=mybir.AluOpType.add)
            rs_t_ps = ps_misc.tile([1, TOK], F32, tag="misc")
            nc.tensor.transpose(rs_t_ps[:], rowsum[:], ident[:])
            rowsum_row = sb.tile([1, TOK], F32)
            nc.vector.tensor_copy(rowsum_row[:], rs_t_ps[:])
            t2_da = sb.tile([1, TOK], F32)
            nc.vector.tensor_scalar(
                out=t2_da[:], in0=sa_sb[:], scalar1=float(N),
                scalar2=sv_tot[:], op0=mybir.AluOpType.mult,
                op1=mybir.AluOpType.subtract)
            da_row = sb.tile([1, TOK], F32)
            nc.vector.scalar_tensor_tensor(
                out=da_row[:], in0=rowsum_row[:], scalar=2.0, in1=t2_da[:],
                op0=mybir.AluOpType.mult, op1=mybir.AluOpType.add)
            dabc_ps = ps_misc.tile([128, TOK], F32, tag="misc")
            nc.tensor.matmul(out=dabc_ps[:], lhsT=scale_row[:], rhs=da_row[:],
                             start=True, stop=True)
            da_bc = sb.tile([128, TOK], F32)
            nc.vector.tensor_copy(da_bc[:], dabc_ps[:])

            # ---------------- dv (ReduceScatter over the 4-core group) ------
            # dvp_raw[i] = 2*sum_j rows[j,i] - 64*sv[i] + SA
            dvr_ps = ps_misc.tile([1, 256], F32, tag="misc")
            nc.tensor.matmul(out=dvr_ps[:], lhsT=ones_f[0:TOK, :],
                             rhs=rows_sb[:], start=True, stop=True)
            dvr_sb = sb.tile([1, 256], F32)
            nc.vector.tensor_copy(dvr_sb[:], dvr_ps[:])
            pay = sb.tile([1, 256], F32)
            nc.vector.scalar_tensor_tensor(
                out=pay[:], in0=dvr_sb[:], scalar=2.0, in1=sv64_sb[:],
                op0=mybir.AluOpType.mult, op1=mybir.AluOpType.subtract)
            dvp_sb = sb.tile([1, 256], F32)
            nc.vector.tensor_scalar(
                out=dvp_sb[:], in0=pay[:], scalar1=sa_tot[:], scalar2=None,
                op0=mybir.AluOpType.add, op1=mybir.AluOpType.bypass)
            rs_in = dram.tile([1, 256], F32)
            rs_out = dram.tile([1, TOK], F32)
            nc.sync.dma_start(rs_in[:], dvp_sb[:])
            if not SKIP_RS:
                nc.gpsimd.collective_compute(
                    "ReduceScatter", mybir.AluOpType.add,
                    replica_groups=[[0, 1, 2, 3], [4, 5, 6, 7]],
                    ins=[rs_in.opt()], outs=[rs_out.opt()],
                )
            else:
                nc.sync.dma_start(rs_out[:], rs_in[:, 0:TOK])
            dv_own = sb.tile([1, TOK], F32)
            nc.sync.dma_start(dv_own[:], rs_out[:])
            dvbc_ps = ps_misc.tile([128, TOK], F32, tag="misc")
            nc.tensor.matmul(out=dvbc_ps[:], lhsT=scale_row[:], rhs=dv_own[:],
                             start=True, stop=True)
            dv_bc = sb.tile([128, TOK], F32)
            nc.vector.tensor_copy(dv_bc[:], dvbc_ps[:])

            # ------- scale + gelu + mm2 + bias + mm3-contribution ----------
            # a-stream first: fully local (hides the ReduceScatter);
            # v-stream after (dv-gated). mm3 accumulates per-stream into one
            # wide PSUM tile.
            o_ps = ps_pe.tile([128, OC, TOK], F32, name="op", tag="pe")                 if not SKIP_MLP else None
            for si, (s, wp, bc, b1off, bmoff) in enumerate((
                ("a", sb_wa, da_bc, 16, 36),
                ("v", sb_wv, dv_bc, 0, 32),
            ) if not SKIP_MLP else ()):
                hsb = sb.tile([128, HC, TOK], BF, name=f"h_{s}")
                for hc in range(HC):
                    sc_ps = ps_dve.tile([128, TOK], F32, name="sc", tag="dve")
                    nc.vector.tensor_mul(sc_ps[:], z_sb[s][:, hc, :], bc[:])
                    nc.scalar.activation(
                        hsb[:, hc, :], sc_ps[:],
                        mybir.ActivationFunctionType.Gelu,
                        bias=sb_gf[:, ds(DC * GF_W + b1off + hc, 1)], scale=1.0,
                    )
                hf = sb.tile([128, DC, TOK], BF, name=f"hf_{s}")
                for dc in range(DC):
                    h2 = ps_pe.tile([128, TOK], F32, name="h2", tag="pe")
                    for hc in range(HC):
                        nc.tensor.matmul(
                            out=h2[:],
                            lhsT=wp[:, ds(WP_WM + hc * 512 + dc * 128, 128)],
                            rhs=hsb[:, hc, :],
                            start=(hc == 0), stop=(hc == HC - 1),
                        )
                    nc.vector.tensor_scalar_add(
                        out=hf[:, dc, :], in0=h2[:],
                        scalar1=sb_gf[:, ds(DC * GF_W + bmoff + dc, 1)])
                for oc in range(OC):
                    for dc in range(DC):
                        nc.tensor.matmul(
                            out=o_ps[:, oc, :],
                            lhsT=wp[:, ds(WP_WO + dc * 512 + oc * 128, 128)],
                            rhs=hf[:, dc, :],
                            start=(si == 0 and oc == 0 and dc == 0),
                            stop=(si == 1 and oc == OC - 1 and dc == DC - 1),
                        )

            # ---------------- bias + output ----------------
            out_sb = sb.tile([128, OC, TOK], F32)
            if SKIP_MLP:
                nc.vector.tensor_copy(out_sb[:, 0, :], dv_bc[:])
            for oc in range(OC if not SKIP_MLP else 0):
                nc.vector.tensor_scalar_add(
                    out=out_sb[:, oc, :], in0=o_ps[:, oc, :],
                    scalar1=sb_gf[:, ds(DC * GF_W + 40 + oc, 1)])
            nc.sync.dma_start(out_d.rearrange("o p t -> p o t"), out_sb[:])

    _split_multi_waits(nc)
    return nc


def _chunk(a, nchunk):
    """[nchunk*128, X] row-major -> [128, nchunk*X] per-partition pack."""
    X = a.shape[1]
    return np.ascontiguousarray(
        a.reshape(nchunk, 128, X).transpose(1, 0, 2).reshape(128, nchunk * X))


def make_in_maps(inputs):
    f32 = np.float32
    x_v = np.asarray(inputs["x_v"], f32)
    x_a = np.asarray(inputs["x_a"], f32)
    W1 = {"v": np.asarray(inputs["W1v"], f32), "a": np.asarray(inputs["W1a"], f32)}
    Wm = {"v": np.asarray(inputs["Wmv"], f32), "a": np.asarray(inputs["Wma"], f32)}
    Wout = np.asarray(inputs["Wout"], f32)
    Wo = {"v": Wout[:D], "a": Wout[D:]}
    b1 = {"v": np.asarray(inputs["b1v"], f32), "a": np.asarray(inputs["b1a"], f32)}
    bm = {"v": np.asarray(inputs["bmv"], f32), "a": np.asarray(inputs["bma"], f32)}
    bout = np.asarray(inputs["bout"], f32)

    wpack = {}
    for s in ("v", "a"):
        wpack[s] = np.concatenate(
            [_chunk(W1[s], DC), _chunk(Wm[s], HC), _chunk(Wo[s], DC)], axis=1
        ).astype(ml_dtypes.bfloat16)

    in_maps = []
    for c in range(NCORES):
        b, g = divmod(c, GROUP)
        sl = slice(g * TOK, (g + 1) * TOK)
        xvT = np.ascontiguousarray(x_v[b].T)  # [D, N]
        xaT = np.ascontiguousarray(x_a[b].T)
        # genpack_bf: per dc: [xvT(256) | xvO(64) | xaO(64)]
        gbf = np.zeros((128, DC, GBF_W), f32)
        gbf[:, :, :256] = xvT.reshape(DC, 128, N).transpose(1, 0, 2)
        gbf[:, :, 256:320] = xvT[:, sl].reshape(DC, 128, TOK).transpose(1, 0, 2)
        gbf[:, :, 320:384] = xaT[:, sl].reshape(DC, 128, TOK).transpose(1, 0, 2)
        gf = np.zeros((128, DC, GF_W), f32)
        xac = xaT[:, sl].reshape(DC, 128, TOK).transpose(1, 0, 2)
        gf[:, :, :64] = xac
        gf[:, :, 64:] = -xac
        bias = np.zeros((128, BIAS_W), f32)
        bias[:, 0:16] = b1["v"].reshape(16, 128).T
        bias[:, 16:32] = b1["a"].reshape(16, 128).T
        bias[:, 32:36] = bm["v"].reshape(4, 128).T
        bias[:, 36:40] = bm["a"].reshape(4, 128).T
        bias[:, 40:44] = bout.reshape(4, 128).T
        in_maps.append({
            "g_bf": np.ascontiguousarray(
                gbf.reshape(128, DC * GBF_W)).astype(ml_dtypes.bfloat16),
            "g_f": np.ascontiguousarray(np.concatenate(
                [gf.reshape(128, DC * GF_W), bias], axis=1)),
            "w_v": wpack["v"],
            "w_a": wpack["a"],
        })
    return in_maps


_CACHE = {}
LAST_PERF = {}


def kernel(**inputs) -> np.ndarray:
    if "nc" not in _CACHE:
        _CACHE["nc"] = build_bass()
    nc = _CACHE["nc"]
    in_maps = make_in_maps(inputs)
    trace = bool(int(os.environ.get("KERNEL_TRACE", "0")))
    if trace:
        try:
            import antenv.axon_hooks  # noqa: F401
        except ModuleNotFoundError:
            trace = False  # axon NTFF hook unavailable in this container
    res = run_bass_kernel_spmd(
        nc, in_maps, core_ids=list(range(NCORES)), has_collectives=True,
        trace=trace,
    )
    LAST_PERF["exec_time_ns"] = res.exec_time_ns
    LAST_PERF["trace"] = res.instructions_and_trace
    out = np.zeros((B, N, D), np.float32)
    for c in range(NCORES):
        b, g = divmod(c, GROUP)
        o = res.results[c]["out"]  # [OC, 128, TOK]
        out[b, g * TOK:(g + 1) * TOK] = o.transpose(2, 0, 1).reshape(TOK, D)
    return out


if __name__ == "__main__":
    # static wait-count validation
    import json
    nc = build_bass()
    bir = json.loads(nc.to_json_bytes())
    bad = 0
    for f in bir["functions"]:
        for blk in f["blocks"]:
            for ins in blk["instructions"]:
                si = ins.get("sync_info") or {}
                ow = si.get("on_wait") or []
                if len(ow) > 1:
                    bad += 1
                    print(f"{ins.get('name')} {ins.get('opcode')}: "
                          f"{len(ow)} waits: {[w.get('ant_name') for w in ow]}")
    print(f"validation: {bad} instructions with >1 wait")


# revision 25
# speedup vs baseline: 1.1119x; 1.1119x over previous
"""Trainium2 Bass kernel for nn_DistanceFusionBlock (retrieval_knn).

Sharding (8 NeuronCores, SPMD single NEFF): token-parallel — core c
handles batch b = c // 4, token quarter g = c % 4 (64 tokens) for BOTH
the v- and a-streams. Inputs arrive host-packed per core (transposed,
chunked, bf16) so no on-device transposes are needed.

Distance phase (the N^2*D part), using |x| = 2*relu(x) - x:
  - 256 gen tiles per core: t = relu(x_v[d,:] - x_a[d,j]) over all 256
    i (free dim), d-chunks on partitions, j in the core's own quarter.
    Split DVE tensor_scalar(sub,max0) [4x mode, 127ns] / ACT Relu with
    per-partition bias [398ns] at ACT_EVERY.
  - The PE folds every tile into row j of a [64,256] PSUM "rows" matrix
    via a sliding one-hot-column lhsT (matmul out base-partition must be
    0/32/64, so scattering is done with the weights, accumulating exact
    zeros elsewhere).
  - sum(diff) corrections are analytic from row/col sums of x_v / x_a
    (tiny PE folds): da_raw[j] = 2*rowsum_j - SV + 256*sa_j (local);
    dv partial = 2*colsum - 64*sv + SA, summed across the 4-core group
    by a 1KB ReduceScatter that also hands each core exactly its own
    64 tokens' slice.

MLP phase: features-on-partitions end-to-end; mm1 runs on RAW inputs
interleaved into the PE fold stream (row scaling commutes:
(dv*x) @ W = dv * (x @ W)); the dv/da scale is applied to the mm1
output (dv broadcast across partitions via a K=1 matmul that also
applies the 1/N), then gelu(+per-partition bias) on ACT, mm2, and the
concat-projection as one wide [128,4,64] PSUM accumulation over both
streams. bf16 operands, fp32 accumulation. The a-stream tail is fully
local and hides the ReduceScatter; only the v-stream tail is dv-gated.

Hardware constraint honored throughout: every TPB instruction has ONE
semaphore wait slot (see _split_multi_waits); per-engine absorber ops
retire each DMA-pack semaphore once so hot-loop ops carry at most one.
"""
import os
import sys

sys.path.insert(0, "/opt/trn_rl_repo")

import numpy as np
import ml_dtypes

import concourse.bass as bass
import concourse.mybir as mybir
import concourse.tile as tile
from concourse.bass import ds
from concourse.bass_utils import run_bass_kernel_spmd

B, N, D, H = 2, 256, 512, 2048
NCORES, GROUP, TOK = 8, 4, 64
DC, HC, OC = D // 128, H // 128, D // 128  # 4, 16, 4
BF, F32 = mybir.dt.bfloat16, mybir.dt.float32
ACT_EVERY = 4  # every ACT_EVERY-th gen tile goes to the scalar engine
SKIP_GEN = False
GEN_BUFS = 8
MM1_BASE_V = 72
MM1_BASE_A = 112
SKIP_MLP = False
SKIP_RS = False

# genpack_bf free-dim layout per d-chunk: [xvT(256) | xvO(64) | xaO(64)]
GBF_W = 384
# genpack_f32 layout per d-chunk: [xa_col(64) | -xa_col(64)]
GF_W = 128
# biaspack layout: [b1v(16) | b1a(16) | bmv(4) | bma(4) | bout(4)]
BIAS_W = 44
# weight pack layout (per stream): [W1(4*2048) | Wm(16*512) | Wout_half(4*512)]
WP_W1, WP_WM, WP_WO = 0, 4 * 2048, 4 * 2048 + 16 * 512
WP_W = WP_WO + 4 * 512  # 18432


def _split_multi_waits(nc):
    """Every TPB instruction struct has exactly ONE semaphore-wait slot;
    this snapshot's Tile doesn't split multi-wait instructions (its wait
    optimizer is disabled). Move all-but-one wait of any instruction onto
    injected same-engine NoOps placed immediately before it."""
    import bass_rust
    n = 0
    for fn in nc.m.functions:
        for blk in fn.blocks:
            out = []
            for ins in blk.instructions:
                si = ins.sync_info
                waits = list(si.on_wait) if si is not None and si.on_wait else []
                if len(waits) > 1:
                    for w in waits[:-1]:
                        nop = bass_rust.InstNoOp(
                            name=f"waitsplit-{n}", engine=ins.engine,
                            ins=[], outs=[])
                        nop.sync_info = mybir.SyncInfo(on_wait=[w], on_update=[])
                        out.append(nop)
                        n += 1
                    si.on_wait = [waits[-1]]
                out.append(ins)
            blk.instructions[:] = out
    return n


def build_bass():
    nc = bass.Bass(num_devices=NCORES)
    g_bf = nc.dram_tensor("g_bf", [128, DC * GBF_W], BF, kind="ExternalInput")
    g_f = nc.dram_tensor("g_f", [128, DC * GF_W + BIAS_W], F32, kind="ExternalInput")
    w_v = nc.dram_tensor("w_v", [128, WP_W], BF, kind="ExternalInput")
    w_a = nc.dram_tensor("w_a", [128, WP_W], BF, kind="ExternalInput")
    out_d = nc.dram_tensor("out", [OC, 128, TOK], F32, kind="ExternalOutput")

    with tile.TileContext(nc) as tc:
        with (
            tc.tile_pool(name="inp", bufs=1) as inp,
            tc.tile_pool(name="gen_d", bufs=GEN_BUFS) as genp_d,
            tc.tile_pool(name="diffp", bufs=3) as diffp,
            tc.tile_pool(name="gen_a", bufs=4) as genp_a,
            tc.tile_pool(name="sb", bufs=1) as sb,
            tc.tile_pool(name="ps_acc", bufs=1, space="PSUM") as ps_acc,
            tc.tile_pool(name="ps_misc", bufs=1, space="PSUM") as ps_misc,
            tc.tile_pool(name="ps_pe", bufs=4, space="PSUM") as ps_pe,
            tc.tile_pool(name="ps_dve", bufs=2, space="PSUM") as ps_dve,
            tc.tile_pool(name="dram", bufs=1, space="DRAM") as dram,
        ):
            # ---------------- input DMAs ----------------
            sb_gbf = inp.tile([128, DC * GBF_W], BF)
            sb_gf = inp.tile([128, DC * GF_W + BIAS_W], F32)
            sb_wv = inp.tile([128, WP_W], BF)
            sb_wa = inp.tile([128, WP_W], BF)
            nc.sync.dma_start(sb_gbf[:], g_bf[:])
            nc.sync.dma_start(sb_gf[:], g_f[:])
            if not SKIP_MLP:
                nc.sync.dma_start(sb_wv[:, ds(WP_W1, WP_WM)], w_v[:, ds(WP_W1, WP_WM)])
                nc.sync.dma_start(sb_wa[:, ds(WP_W1, WP_WM)], w_a[:, ds(WP_W1, WP_WM)])
                nc.sync.dma_start(sb_wv[:, ds(WP_WM, WP_W - WP_WM)], w_v[:, ds(WP_WM, WP_W - WP_WM)])
                nc.sync.dma_start(sb_wa[:, ds(WP_WM, WP_W - WP_WM)], w_a[:, ds(WP_WM, WP_W - WP_WM)])
            else:
                nc.sync.dma_start(sb_wv[:, 0:2], w_v[:, 0:2])
                nc.sync.dma_start(sb_wa[:, 0:2], w_a[:, 0:2])

            # ---------------- constants ----------------
            zeros = sb.tile([128, 256], BF)
            ones_bf = sb.tile([128, 1], BF)
            ones_f = sb.tile([128, 1], F32)
            c64_bf = sb.tile([128, 1], BF)
            c2_f = sb.tile([128, 1], F32)
            scale_row = sb.tile([1, 128], F32)
            zo = sb.tile([128, 128], BF)  # single ones-column at index TOK
            ident = sb.tile([TOK, TOK], F32)
            nc.vector.memset(zeros[:], 0.0)
            nc.vector.memset(ones_bf[:], 1.0)
            nc.vector.memset(ones_f[:], 1.0)
            nc.vector.memset(c64_bf[:], float(TOK))
            nc.vector.memset(c2_f[:], 2.0)
            nc.vector.memset(scale_row[:], 1.0 / N)
            nc.vector.memset(zo[:], 0.0)
            nc.vector.memset(zo[:, TOK:TOK + 1], 1.0)
            from concourse.masks import make_identity
            make_identity(nc, ident[:])

            # ---------------- per-engine semaphore absorbers ----------------
            # DVE: touch each DMA pack once (1 wait per op, dataflow-safe by
            # priority order).
            dve_scr = sb.tile([1, 2], F32)
            nc.vector.tensor_copy(dve_scr[0:1, 0:1], sb_gf[0:1, 0:1])
            dve_scr2 = sb.tile([1, 2], BF)
            nc.vector.tensor_copy(dve_scr2[0:1, 0:1], sb_gbf[0:1, 0:1])
            # ACT: same, plus warm the gelu/abs table set early.
            act_scr = sb.tile([1, 2], BF)
            nc.scalar.copy(act_scr[0:1, 0:1], sb_gbf[0:1, 0:1])
            act_scr2 = sb.tile([1, 2], F32)
            nc.scalar.copy(act_scr2[0:1, 0:1], sb_gf[0:1, 0:1])
            warm = sb.tile([128, 1], BF)
            nc.scalar.activation(warm[:], zeros[:, 0:1],
                                 mybir.ActivationFunctionType.Gelu)
            # PE: dummy 1-col matmuls absorbing each pack's semaphore.
            scr_ps = ps_misc.tile([1, 1], F32, tag="misc")
            nc.tensor.matmul(out=scr_ps[:], lhsT=ones_bf[:], rhs=ones_bf[:],
                             start=True, stop=True)
            scr_ps2 = ps_misc.tile([1, 1], F32, name="scr2", tag="misc")
            nc.tensor.matmul(out=scr_ps2[:], lhsT=ones_bf[:],
                             rhs=sb_gbf[:, 0:1], start=True, stop=True)

            # ---------------- distance phase ----------------
            # relu trick: |x| = 2*relu(x) - x, and sum(diff) is analytic.
            # Each tile t = relu(x_v[d,:] - x_a[d,j]); folds write row j of
            # rows_ps via a sliding one-hot column lhsT.
            # sv/sa ingredient folds first (their DVE tail overlaps gen)
            sv64_ps = ps_misc.tile([1, 256], F32, tag="misc")
            for dc in range(DC):
                nc.tensor.matmul(out=sv64_ps[:], lhsT=c64_bf[:],
                                 rhs=sb_gbf[:, ds(dc * GBF_W, 256)],
                                 start=(dc == 0), stop=(dc == DC - 1))
            sv64_sb = sb.tile([1, 256], F32)
            nc.vector.tensor_copy(sv64_sb[:], sv64_ps[:])
            sa_ps = ps_misc.tile([1, TOK], F32, tag="misc")
            for dc in range(DC):
                nc.tensor.matmul(out=sa_ps[:], lhsT=ones_bf[:],
                                 rhs=sb_gbf[:, ds(dc * GBF_W + 320, TOK)],
                                 start=(dc == 0), stop=(dc == DC - 1))
            sa_sb = sb.tile([1, TOK], F32)
            nc.vector.tensor_copy(sa_sb[:], sa_ps[:])
            sa_tot = sb.tile([1, 1], F32)
            nc.vector.tensor_reduce(sa_tot[:], sa_sb[:],
                                    axis=mybir.AxisListType.X,
                                    op=mybir.AluOpType.add)
            svq = sb.tile([1, 1], F32)
            nc.vector.tensor_reduce(svq[:], sv64_sb[:],
                                    axis=mybir.AxisListType.X,
                                    op=mybir.AluOpType.add)
            sv_tot = sb.tile([1, 1], F32)
            nc.vector.tensor_scalar(
                out=sv_tot[:], in0=svq[:], scalar1=1.0 / TOK, scalar2=None,
                op0=mybir.AluOpType.mult, op1=mybir.AluOpType.bypass)
            rows_ps = ps_acc.tile([TOK, 256], F32)
            njj = TOK if not SKIP_GEN else 1
            nfold = njj * DC
            k = 0
            for j in range(njj):
                for dc in range(DC):
                    use_act = k % ACT_EVERY == ACT_EVERY - 1
                    t = (genp_a if use_act else genp_d).tile(
                        [128, 256], BF, name="gt")
                    if use_act:
                        nc.scalar.activation(
                            t[:], sb_gbf[:, ds(dc * GBF_W, 256)],
                            mybir.ActivationFunctionType.Relu,
                            bias=sb_gf[:, ds(dc * GF_W + 64 + j, 1)],
                            scale=1.0,
                        )
                    else:
                        nc.vector.tensor_scalar(
                            out=t[:],
                            in0=sb_gbf[:, ds(dc * GBF_W, 256)],
                            scalar1=sb_gf[:, ds(dc * GF_W + j, 1)],
                            scalar2=0.0,
                            op0=mybir.AluOpType.subtract,
                            op1=mybir.AluOpType.max,
                        )
                    nc.tensor.matmul(
                        out=rows_ps[:], lhsT=zo[:, ds(TOK - j, TOK)],
                        rhs=t[:], start=(k == 0), stop=(k == nfold - 1))
                    k += 1
            # rows -> SBUF
            rows_sb = sb.tile([TOK, 256], F32)
            nc.vector.tensor_copy(rows_sb[:], rows_ps[:])

            # -------- dv payload + ReduceScatter dispatch (ASAP) ----------
            # payload[i] = 2*colsum(rows)[i] - 64*sv[i] + SA, fused:
            # colsum folds the 2x into the weights; one STT adds SA and
            # subtracts 64*sv.
            dvr_ps = ps_misc.tile([1, 256], F32, tag="misc")
            nc.tensor.matmul(out=dvr_ps[:], lhsT=c2_f[0:TOK, :],
                             rhs=rows_sb[:], start=True, stop=True)
            dvr_sb = sb.tile([1, 256], F32)
            nc.vector.tensor_copy(dvr_sb[:], dvr_ps[:])
            dvp_sb = sb.tile([1, 256], F32)
            nc.vector.scalar_tensor_tensor(
                out=dvp_sb[:], in0=dvr_sb[:], scalar=sa_tot[:],
                in1=sv64_sb[:], op0=mybir.AluOpType.add,
                op1=mybir.AluOpType.subtract)
            rs_in = dram.tile([1, 256], F32)
            rs_out = dram.tile([1, TOK], F32)
            nc.sync.dma_start(rs_in[:], dvp_sb[:])
            if not SKIP_RS:
                nc.gpsimd.collective_compute(
                    "ReduceScatter", mybir.AluOpType.add,
                    replica_groups=[[0, 1, 2, 3], [4, 5, 6, 7]],
                    ins=[rs_in.opt()], outs=[rs_out.opt()],
                )
            else:
                nc.sync.dma_start(rs_out[:], rs_in[:, 0:TOK])


            # ---------------- da (local, from rows + analytic corr) -------
            # da_raw[j] = 2*sum_i rows[j,i] - SV + 256*sa[j]
            rowsum = sb.tile([TOK, 1], F32)
            nc.vector.tensor_reduce(rowsum[:], rows_sb[:],
                                    axis=mybir.AxisListType.X,
                                    op=mybir.AluOpType.add)
            rs_t_ps = ps_misc.tile([1, TOK], F32, tag="misc")
            nc.tensor.transpose(rs_t_ps[:], rowsum[:], ident[:])
            rowsum_row = sb.tile([1, TOK], F32)
            nc.vector.tensor_copy(rowsum_row[:], rs_t_ps[:])
            t2_da = sb.tile([1, TOK], F32)
            nc.vector.tensor_scalar(
                out=t2_da[:], in0=sa_sb[:], scalar1=float(N),
                scalar2=sv_tot[:], op0=mybir.AluOpType.mult,
                op1=mybir.AluOpType.subtract)
            da_row = sb.tile([1, TOK], F32)
            nc.vector.scalar_tensor_tensor(
                out=da_row[:], in0=rowsum_row[:], scalar=2.0, in1=t2_da[:],
                op0=mybir.AluOpType.mult, op1=mybir.AluOpType.add)
            dabc_ps = ps_misc.tile([128, TOK], F32, tag="misc")
            nc.tensor.matmul(out=dabc_ps[:], lhsT=scale_row[:], rhs=da_row[:],
                             start=True, stop=True)
            da_bc = sb.tile([128, TOK], F32)
            nc.vector.tensor_copy(da_bc[:], dabc_ps[:])

            # ---------------- mm1 raw (both streams; overlaps the RS) -------
            z_sb = {}
            for s, wp, xoff in ((("v", sb_wv, 256), ("a", sb_wa, 320))
                                if not SKIP_MLP else ()):
                z_sb[s] = sb.tile([128, HC, TOK], BF, name=f"z_{s}")
                for grp in range(HC // 4):
                    zp = ps_pe.tile([128, 4, TOK], F32, name="zp", tag="pe")
                    for hcm in range(4):
                        hc = grp * 4 + hcm
                        for dcw in range(DC):
                            nc.tensor.matmul(
                                out=zp[:, hcm, :],
                                lhsT=wp[:, ds(WP_W1 + dcw * 2048 + hc * 128, 128)],
                                rhs=sb_gbf[:, ds(dcw * GBF_W + xoff, TOK)],
                                start=(dcw == 0), stop=(dcw == DC - 1),
                            )
                    nc.vector.tensor_copy(z_sb[s][:, ds(grp * 4, 4), :], zp[:])

            # ---------------- dv readback ----------------
            dv_own = sb.tile([1, TOK], F32)
            nc.sync.dma_start(dv_own[:], rs_out[:])
            dvbc_ps = ps_misc.tile([128, TOK], F32, tag="misc")
            nc.tensor.matmul(out=dvbc_ps[:], lhsT=scale_row[:], rhs=dv_own[:],
                             start=True, stop=True)
            dv_bc = sb.tile([128, TOK], F32)
            nc.vector.tensor_copy(dv_bc[:], dvbc_ps[:])

            # ------- scale + gelu + mm2 + bias + mm3-contribution ----------
            # a-stream first: fully local (hides the ReduceScatter);
            # v-stream after (dv-gated). mm3 accumulates per-stream into one
            # wide PSUM tile.
            o_ps = ps_pe.tile([128, OC, TOK], F32, name="op", tag="pe")                 if not SKIP_MLP else None
            for si, (s, wp, bc, b1off, bmoff) in enumerate((
                ("a", sb_wa, da_bc, 16, 36),
                ("v", sb_wv, dv_bc, 0, 32),
            ) if not SKIP_MLP else ()):
                hsb = sb.tile([128, HC, TOK], BF, name=f"h_{s}")
                sc_sb = sb.tile([128, HC, TOK], BF, name=f"sc_{s}")
                for hc in range(HC):
                    nc.vector.tensor_mul(sc_sb[:, hc, :], z_sb[s][:, hc, :], bc[:])
                for hc in range(HC):
                    nc.scalar.activation(
                        hsb[:, hc, :], sc_sb[:, hc, :],
                        mybir.ActivationFunctionType.Gelu,
                        bias=sb_gf[:, ds(DC * GF_W + b1off + hc, 1)], scale=1.0,
                    )
                hf = sb.tile([128, DC, TOK], BF, name=f"hf_{s}")
                for dc in range(DC):
                    h2 = ps_pe.tile([128, TOK], F32, name="h2", tag="pe")
                    for hc in range(HC):
                        nc.tensor.matmul(
                            out=h2[:],
                            lhsT=wp[:, ds(WP_WM + hc * 512 + dc * 128, 128)],
                            rhs=hsb[:, hc, :],
                            start=(hc == 0), stop=(hc == HC - 1),
                        )
                    nc.vector.tensor_scalar_add(
                        out=hf[:, dc, :], in0=h2[:],
                        scalar1=sb_gf[:, ds(DC * GF_W + bmoff + dc, 1)])
                for oc in range(OC):
                    for dc in range(DC):
                        nc.tensor.matmul(
                            out=o_ps[:, oc, :],
                            lhsT=wp[:, ds(WP_WO + dc * 512 + oc * 128, 128)],
                            rhs=hf[:, dc, :],
                            start=(si == 0 and oc == 0 and dc == 0),
                            stop=(si == 1 and oc == OC - 1 and dc == DC - 1),
                        )

            # ---------------- bias + output ----------------
            out_sb = sb.tile([128, OC, TOK], F32)
            if SKIP_MLP:
                nc.vector.tensor_copy(out_sb[:, 0, :], dv_bc[:])
            for oc in range(OC if not SKIP_MLP else 0):
                nc.vector.tensor_scalar_add(
                    out=out_sb[:, oc, :], in0=o_ps[:, oc, :],
                    scalar1=sb_gf[:, ds(DC * GF_W + 40 + oc, 1)])
            nc.sync.dma_start(out_d.rearrange("o p t -> p o t"), out_sb[:])

    _split_multi_waits(nc)
    return nc


def _chunk(a, nchunk):
    """[nchunk*128, X] row-major -> [128, nchunk*X] per-partition pack."""
    X = a.shape[1]
    return np.ascontiguousarray(
        a.reshape(nchunk, 128, X).transpose(1, 0, 2).reshape(128, nchunk * X))


def make_in_maps(inputs):
    f32 = np.float32
    x_v = np.asarray(inputs["x_v"], f32)
    x_a = np.asarray(inputs["x_a"], f32)
    W1 = {"v": np.asarray(inputs["W1v"], f32), "a": np.asarray(inputs["W1a"], f32)}
    Wm = {"v": np.asarray(inputs["Wmv"], f32), "a": np.asarray(inputs["Wma"], f32)}
    Wout = np.asarray(inputs["Wout"], f32)
    Wo = {"v": Wout[:D], "a": Wout[D:]}
    b1 = {"v": np.asarray(inputs["b1v"], f32), "a": np.asarray(inputs["b1a"], f32)}
    bm = {"v": np.asarray(inputs["bmv"], f32), "a": np.asarray(inputs["bma"], f32)}
    bout = np.asarray(inputs["bout"], f32)

    wpack = {}
    for s in ("v", "a"):
        wpack[s] = np.concatenate(
            [_chunk(W1[s], DC), _chunk(Wm[s], HC), _chunk(Wo[s], DC)], axis=1
        ).astype(ml_dtypes.bfloat16)

    in_maps = []
    for c in range(NCORES):
        b, g = divmod(c, GROUP)
        sl = slice(g * TOK, (g + 1) * TOK)
        xvT = np.ascontiguousarray(x_v[b].T)  # [D, N]
        xaT = np.ascontiguousarray(x_a[b].T)
        # genpack_bf: per dc: [xvT(256) | xvO(64) | xaO(64)]
        gbf = np.zeros((128, DC, GBF_W), f32)
        gbf[:, :, :256] = xvT.reshape(DC, 128, N).transpose(1, 0, 2)
        gbf[:, :, 256:320] = xvT[:, sl].reshape(DC, 128, TOK).transpose(1, 0, 2)
        gbf[:, :, 320:384] = xaT[:, sl].reshape(DC, 128, TOK).transpose(1, 0, 2)
        gf = np.zeros((128, DC, GF_W), f32)
        xac = xaT[:, sl].reshape(DC, 128, TOK).transpose(1, 0, 2)
        gf[:, :, :64] = xac
        gf[:, :, 64:] = -xac
        bias = np.zeros((128, BIAS_W), f32)
        bias[:, 0:16] = b1["v"].reshape(16, 128).T
        bias[:, 16:32] = b1["a"].reshape(16, 128).T
        bias[:, 32:36] = bm["v"].reshape(4, 128).T
        bias[:, 36:40] = bm["a"].reshape(4, 128).T
        bias[:, 40:44] = bout.reshape(4, 128).T
        in_maps.append({
            "g_bf": np.ascontiguousarray(
                gbf.reshape(128, DC * GBF_W)).astype(ml_dtypes.bfloat16),
            "g_f": np.ascontiguousarray(np.concatenate(
                [gf.reshape(128, DC * GF_W), bias], axis=1)),
            "w_v": wpack["v"],
            "w_a": wpack["a"],
        })
    return in_maps


_CACHE = {}
LAST_PERF = {}


def kernel(**inputs) -> np.ndarray:
    if "nc" not in _CACHE:
        _CACHE["nc"] = build_bass()
    nc = _CACHE["nc"]
    in_maps = make_in_maps(inputs)
    trace = bool(int(os.environ.get("KERNEL_TRACE", "0")))
    if trace:
        try:
            import antenv.axon_hooks  # noqa: F401
        except ModuleNotFoundError:
            trace = False  # axon NTFF hook unavailable in this container
    res = run_bass_kernel_spmd(
        nc, in_maps, core_ids=list(range(NCORES)), has_collectives=True,
        trace=trace,
    )
    LAST_PERF["exec_time_ns"] = res.exec_time_ns
    LAST_PERF["trace"] = res.instructions_and_trace
    out = np.zeros((B, N, D), np.float32)
    for c in range(NCORES):
        b, g = divmod(c, GROUP)
        o = res.results[c]["out"]  # [OC, 128, TOK]
        out[b, g * TOK:(g + 1) * TOK] = o.transpose(2, 0, 1).reshape(TOK, D)
    return out


if __name__ == "__main__":
    # static wait-count validation
    import json
    nc = build_bass()
    bir = json.loads(nc.to_json_bytes())
    bad = 0
    for f in bir["functions"]:
        for blk in f["blocks"]:
            for ins in blk["instructions"]:
                si = ins.get("sync_info") or {}
                ow = si.get("on_wait") or []
                if len(ow) > 1:
                    bad += 1
                    print(f"{ins.get('name')} {ins.get('opcode')}: "
                          f"{len(ow)} waits: {[w.get('ant_name') for w in ow]}")
    print(f"validation: {bad} instructions with >1 wait")


# revision 26
# speedup vs baseline: 1.1280x; 1.0145x over previous
"""Trainium2 Bass kernel for nn_DistanceFusionBlock (retrieval_knn).

Sharding (8 NeuronCores, SPMD single NEFF): token-parallel — core c
handles batch b = c // 4, token quarter g = c % 4 (64 tokens) for BOTH
the v- and a-streams. Inputs arrive host-packed per core (transposed,
chunked, bf16) so no on-device transposes are needed.

Distance phase (the N^2*D part), using |x| = 2*relu(x) - x:
  - 256 gen tiles per core: t = relu(x_v[d,:] - x_a[d,j]) over all 256
    i (free dim), d-chunks on partitions, j in the core's own quarter.
    Split DVE tensor_scalar(sub,max0) [4x mode, 127ns] / ACT Relu with
    per-partition bias [398ns] at ACT_EVERY.
  - The PE folds every tile into row j of a [64,256] PSUM "rows" matrix
    via a sliding one-hot-column lhsT (matmul out base-partition must be
    0/32/64, so scattering is done with the weights, accumulating exact
    zeros elsewhere).
  - sum(diff) corrections are analytic from row/col sums of x_v / x_a
    (tiny PE folds): da_raw[j] = 2*rowsum_j - SV + 256*sa_j (local);
    dv partial = 2*colsum - 64*sv + SA, summed across the 4-core group
    by a 1KB ReduceScatter that also hands each core exactly its own
    64 tokens' slice.

MLP phase: features-on-partitions end-to-end; mm1 runs on RAW inputs
interleaved into the PE fold stream (row scaling commutes:
(dv*x) @ W = dv * (x @ W)); the dv/da scale is applied to the mm1
output (dv broadcast across partitions via a K=1 matmul that also
applies the 1/N), then gelu(+per-partition bias) on ACT, mm2, and the
concat-projection as one wide [128,4,64] PSUM accumulation over both
streams. bf16 operands, fp32 accumulation. The a-stream tail is fully
local and hides the ReduceScatter; only the v-stream tail is dv-gated.

Hardware constraint honored throughout: every TPB instruction has ONE
semaphore wait slot (see _split_multi_waits); per-engine absorber ops
retire each DMA-pack semaphore once so hot-loop ops carry at most one.
"""
import os
import sys

sys.path.insert(0, "/opt/trn_rl_repo")

import numpy as np
import ml_dtypes

import concourse.bass as bass
import concourse.mybir as mybir
import concourse.tile as tile
from concourse.bass import ds
from concourse.bass_utils import run_bass_kernel_spmd

B, N, D, H = 2, 256, 512, 2048
NCORES, GROUP, TOK = 8, 4, 64
DC, HC, OC = D // 128, H // 128, D // 128  # 4, 16, 4
BF, F32 = mybir.dt.bfloat16, mybir.dt.float32
ACT_EVERY = 4  # every ACT_EVERY-th gen tile goes to the scalar engine
SKIP_GEN = False
GEN_BUFS = 8
MM1_BASE_V = 72
MM1_BASE_A = 112
SKIP_MLP = False
SKIP_RS = False

# genpack_bf free-dim layout per d-chunk: [xvT(256) | xvO(64) | xaO(64)]
GBF_W = 384
# genpack_f32 layout per d-chunk: [xa_col(64) | -xa_col(64)]
GF_W = 128
# biaspack layout: [b1v(16) | b1a(16) | bmv(4) | bma(4) | bout(4)]
BIAS_W = 44
# weight pack layout (per stream): [W1(4*2048) | Wm(16*512) | Wout_half(4*512)]
WP_W1, WP_WM, WP_WO = 0, 4 * 2048, 4 * 2048 + 16 * 512
WP_W = WP_WO + 4 * 512  # 18432


def _split_multi_waits(nc):
    """Every TPB instruction struct has exactly ONE semaphore-wait slot;
    this snapshot's Tile doesn't split multi-wait instructions (its wait
    optimizer is disabled). Move all-but-one wait of any instruction onto
    injected same-engine NoOps placed immediately before it."""
    import bass_rust
    n = 0
    for fn in nc.m.functions:
        for blk in fn.blocks:
            out = []
            for ins in blk.instructions:
                si = ins.sync_info
                waits = list(si.on_wait) if si is not None and si.on_wait else []
                if len(waits) > 1:
                    for w in waits[:-1]:
                        nop = bass_rust.InstNoOp(
                            name=f"waitsplit-{n}", engine=ins.engine,
                            ins=[], outs=[])
                        nop.sync_info = mybir.SyncInfo(on_wait=[w], on_update=[])
                        out.append(nop)
                        n += 1
                    si.on_wait = [waits[-1]]
                out.append(ins)
            blk.instructions[:] = out
    return n


def build_bass():
    nc = bass.Bass(num_devices=NCORES)
    g_bf = nc.dram_tensor("g_bf", [128, DC * GBF_W], BF, kind="ExternalInput")
    g_f = nc.dram_tensor("g_f", [128, DC * GF_W + BIAS_W], F32, kind="ExternalInput")
    w_v = nc.dram_tensor("w_v", [128, WP_W], BF, kind="ExternalInput")
    w_a = nc.dram_tensor("w_a", [128, WP_W], BF, kind="ExternalInput")
    out_d = nc.dram_tensor("out", [OC, 128, TOK], F32, kind="ExternalOutput")

    with tile.TileContext(nc) as tc:
        with (
            tc.tile_pool(name="inp", bufs=1) as inp,
            tc.tile_pool(name="gen_d", bufs=GEN_BUFS) as genp_d,
            tc.tile_pool(name="diffp", bufs=3) as diffp,
            tc.tile_pool(name="gen_a", bufs=4) as genp_a,
            tc.tile_pool(name="sb", bufs=1) as sb,
            tc.tile_pool(name="ps_acc", bufs=1, space="PSUM") as ps_acc,
            tc.tile_pool(name="ps_misc", bufs=1, space="PSUM") as ps_misc,
            tc.tile_pool(name="ps_pe", bufs=4, space="PSUM") as ps_pe,
            tc.tile_pool(name="ps_dve", bufs=2, space="PSUM") as ps_dve,
            tc.tile_pool(name="dram", bufs=1, space="DRAM") as dram,
        ):
            # ---------------- input DMAs ----------------
            sb_gbf = inp.tile([128, DC * GBF_W], BF)
            sb_gf = inp.tile([128, DC * GF_W + BIAS_W], F32)
            sb_wv = inp.tile([128, WP_W], BF)
            sb_wa = inp.tile([128, WP_W], BF)
            nc.sync.dma_start(sb_gf[:], g_f[:])
            for dc in range(DC):
                nc.sync.dma_start(sb_gbf[:, ds(dc * GBF_W, GBF_W)],
                                  g_bf[:, ds(dc * GBF_W, GBF_W)])
            if not SKIP_MLP:
                nc.sync.dma_start(sb_wv[:, ds(WP_W1, WP_WM)], w_v[:, ds(WP_W1, WP_WM)])
                nc.sync.dma_start(sb_wa[:, ds(WP_W1, WP_WM)], w_a[:, ds(WP_W1, WP_WM)])
                nc.sync.dma_start(sb_wv[:, ds(WP_WM, WP_W - WP_WM)], w_v[:, ds(WP_WM, WP_W - WP_WM)])
                nc.sync.dma_start(sb_wa[:, ds(WP_WM, WP_W - WP_WM)], w_a[:, ds(WP_WM, WP_W - WP_WM)])
            else:
                nc.sync.dma_start(sb_wv[:, 0:2], w_v[:, 0:2])
                nc.sync.dma_start(sb_wa[:, 0:2], w_a[:, 0:2])

            # ---------------- constants ----------------
            zeros = sb.tile([128, 256], BF)
            ones_bf = sb.tile([128, 1], BF)
            ones_f = sb.tile([128, 1], F32)
            c64_bf = sb.tile([128, 1], BF)
            c2_f = sb.tile([128, 1], F32)
            scale_row = sb.tile([1, 128], F32)
            zo = sb.tile([128, 128], BF)  # single ones-column at index TOK
            ident = sb.tile([TOK, TOK], F32)
            nc.vector.memset(zeros[:], 0.0)
            nc.vector.memset(ones_bf[:], 1.0)
            nc.vector.memset(ones_f[:], 1.0)
            nc.vector.memset(c64_bf[:], float(TOK))
            nc.vector.memset(c2_f[:], 2.0)
            nc.vector.memset(scale_row[:], 1.0 / N)
            nc.vector.memset(zo[:], 0.0)
            nc.vector.memset(zo[:, TOK:TOK + 1], 1.0)
            from concourse.masks import make_identity
            make_identity(nc, ident[:])

            # ---------------- per-engine semaphore absorbers ----------------
            # DVE: touch each DMA pack once (1 wait per op, dataflow-safe by
            # priority order).
            dve_scr = sb.tile([1, 2], F32)
            nc.vector.tensor_copy(dve_scr[0:1, 0:1], sb_gf[0:1, 0:1])
            dve_scr2 = sb.tile([1, 2], BF)
            nc.vector.tensor_copy(dve_scr2[0:1, 0:1], sb_gbf[0:1, 0:1])
            # ACT: same, plus warm the gelu/abs table set early.
            act_scr = sb.tile([1, 2], BF)
            nc.scalar.copy(act_scr[0:1, 0:1], sb_gbf[0:1, 0:1])
            act_scr2 = sb.tile([1, 2], F32)
            nc.scalar.copy(act_scr2[0:1, 0:1], sb_gf[0:1, 0:1])
            warm = sb.tile([128, 1], BF)
            nc.scalar.activation(warm[:], zeros[:, 0:1],
                                 mybir.ActivationFunctionType.Gelu)
            # PE: dummy 1-col matmuls absorbing each pack's semaphore.
            scr_ps = ps_misc.tile([1, 1], F32, tag="misc")
            nc.tensor.matmul(out=scr_ps[:], lhsT=ones_bf[:], rhs=ones_bf[:],
                             start=True, stop=True)
            scr_ps2 = ps_misc.tile([1, 1], F32, name="scr2", tag="misc")
            nc.tensor.matmul(out=scr_ps2[:], lhsT=ones_bf[:],
                             rhs=sb_gbf[:, 0:1], start=True, stop=True)

            # ---------------- distance phase ----------------
            # relu trick: |x| = 2*relu(x) - x, and sum(diff) is analytic.
            # Each tile t = relu(x_v[d,:] - x_a[d,j]); folds write row j of
            # rows_ps via a sliding one-hot column lhsT.
            # sv/sa ingredient folds first (their DVE tail overlaps gen)
            sv64_ps = ps_misc.tile([1, 256], F32, tag="misc")
            for dc in range(DC):
                nc.tensor.matmul(out=sv64_ps[:], lhsT=c64_bf[:],
                                 rhs=sb_gbf[:, ds(dc * GBF_W, 256)],
                                 start=(dc == 0), stop=(dc == DC - 1))
            sv64_sb = sb.tile([1, 256], F32)
            nc.vector.tensor_copy(sv64_sb[:], sv64_ps[:])
            sa_ps = ps_misc.tile([1, TOK], F32, tag="misc")
            for dc in range(DC):
                nc.tensor.matmul(out=sa_ps[:], lhsT=ones_bf[:],
                                 rhs=sb_gbf[:, ds(dc * GBF_W + 320, TOK)],
                                 start=(dc == 0), stop=(dc == DC - 1))
            sa_sb = sb.tile([1, TOK], F32)
            nc.vector.tensor_copy(sa_sb[:], sa_ps[:])
            sa_tot = sb.tile([1, 1], F32)
            nc.vector.tensor_reduce(sa_tot[:], sa_sb[:],
                                    axis=mybir.AxisListType.X,
                                    op=mybir.AluOpType.add)
            svq = sb.tile([1, 1], F32)
            nc.vector.tensor_reduce(svq[:], sv64_sb[:],
                                    axis=mybir.AxisListType.X,
                                    op=mybir.AluOpType.add)
            sv_tot = sb.tile([1, 1], F32)
            nc.vector.tensor_scalar(
                out=sv_tot[:], in0=svq[:], scalar1=1.0 / TOK, scalar2=None,
                op0=mybir.AluOpType.mult, op1=mybir.AluOpType.bypass)
            rows_ps = ps_acc.tile([TOK, 256], F32)
            njj = TOK if not SKIP_GEN else 1
            nfold = njj * DC
            k = 0
            for dc in range(DC):
                for j in range(njj):
                    use_act = k % ACT_EVERY == ACT_EVERY - 1
                    t = (genp_a if use_act else genp_d).tile(
                        [128, 256], BF, name="gt")
                    if use_act:
                        nc.scalar.activation(
                            t[:], sb_gbf[:, ds(dc * GBF_W, 256)],
                            mybir.ActivationFunctionType.Relu,
                            bias=sb_gf[:, ds(dc * GF_W + 64 + j, 1)],
                            scale=1.0,
                        )
                    else:
                        nc.vector.tensor_scalar(
                            out=t[:],
                            in0=sb_gbf[:, ds(dc * GBF_W, 256)],
                            scalar1=sb_gf[:, ds(dc * GF_W + j, 1)],
                            scalar2=0.0,
                            op0=mybir.AluOpType.subtract,
                            op1=mybir.AluOpType.max,
                        )
                    nc.tensor.matmul(
                        out=rows_ps[:], lhsT=zo[:, ds(TOK - j, TOK)],
                        rhs=t[:], start=(k == 0), stop=(k == nfold - 1))
                    k += 1
            # rows -> SBUF
            rows_sb = sb.tile([TOK, 256], F32)
            nc.vector.tensor_copy(rows_sb[:], rows_ps[:])

            # -------- dv payload + ReduceScatter dispatch (ASAP) ----------
            # payload[i] = 2*colsum(rows)[i] - 64*sv[i] + SA, fused:
            # colsum folds the 2x into the weights; one STT adds SA and
            # subtracts 64*sv.
            dvr_ps = ps_misc.tile([1, 256], F32, tag="misc")
            nc.tensor.matmul(out=dvr_ps[:], lhsT=c2_f[0:TOK, :],
                             rhs=rows_sb[:], start=True, stop=True)
            dvr_sb = sb.tile([1, 256], F32)
            nc.vector.tensor_copy(dvr_sb[:], dvr_ps[:])
            dvp_sb = sb.tile([1, 256], F32)
            nc.vector.scalar_tensor_tensor(
                out=dvp_sb[:], in0=dvr_sb[:], scalar=sa_tot[:],
                in1=sv64_sb[:], op0=mybir.AluOpType.add,
                op1=mybir.AluOpType.subtract)
            rs_in = dram.tile([1, 256], F32)
            rs_out = dram.tile([1, TOK], F32)
            nc.sync.dma_start(rs_in[:], dvp_sb[:])
            if not SKIP_RS:
                nc.gpsimd.collective_compute(
                    "ReduceScatter", mybir.AluOpType.add,
                    replica_groups=[[0, 1, 2, 3], [4, 5, 6, 7]],
                    ins=[rs_in.opt()], outs=[rs_out.opt()],
                )
            else:
                nc.sync.dma_start(rs_out[:], rs_in[:, 0:TOK])


            # ---------------- da (local, from rows + analytic corr) -------
            # da_raw[j] = 2*sum_i rows[j,i] - SV + 256*sa[j]
            rowsum = sb.tile([TOK, 1], F32)
            nc.vector.tensor_reduce(rowsum[:], rows_sb[:],
                                    axis=mybir.AxisListType.X,
                                    op=mybir.AluOpType.add)
            rs_t_ps = ps_misc.tile([1, TOK], F32, tag="misc")
            nc.tensor.transpose(rs_t_ps[:], rowsum[:], ident[:])
            rowsum_row = sb.tile([1, TOK], F32)
            nc.vector.tensor_copy(rowsum_row[:], rs_t_ps[:])
            t2_da = sb.tile([1, TOK], F32)
            nc.vector.tensor_scalar(
                out=t2_da[:], in0=sa_sb[:], scalar1=float(N),
                scalar2=sv_tot[:], op0=mybir.AluOpType.mult,
                op1=mybir.AluOpType.subtract)
            da_row = sb.tile([1, TOK], F32)
            nc.vector.scalar_tensor_tensor(
                out=da_row[:], in0=rowsum_row[:], scalar=2.0, in1=t2_da[:],
                op0=mybir.AluOpType.mult, op1=mybir.AluOpType.add)
            dabc_ps = ps_misc.tile([128, TOK], F32, tag="misc")
            nc.tensor.matmul(out=dabc_ps[:], lhsT=scale_row[:], rhs=da_row[:],
                             start=True, stop=True)
            da_bc = sb.tile([128, TOK], F32)
            nc.vector.tensor_copy(da_bc[:], dabc_ps[:])

            # ---------------- mm1 raw (both streams; overlaps the RS) -------
            z_sb = {}
            for s, wp, xoff in ((("v", sb_wv, 256), ("a", sb_wa, 320))
                                if not SKIP_MLP else ()):
                z_sb[s] = sb.tile([128, HC, TOK], BF, name=f"z_{s}")
                for grp in range(HC // 4):
                    zp = ps_pe.tile([128, 4, TOK], F32, name="zp", tag="pe")
                    for hcm in range(4):
                        hc = grp * 4 + hcm
                        for dcw in range(DC):
                            nc.tensor.matmul(
                                out=zp[:, hcm, :],
                                lhsT=wp[:, ds(WP_W1 + dcw * 2048 + hc * 128, 128)],
                                rhs=sb_gbf[:, ds(dcw * GBF_W + xoff, TOK)],
                                start=(dcw == 0), stop=(dcw == DC - 1),
                            )
                    nc.vector.tensor_copy(z_sb[s][:, ds(grp * 4, 4), :], zp[:])

            # ---------------- dv readback ----------------
            dv_own = sb.tile([1, TOK], F32)
            nc.sync.dma_start(dv_own[:], rs_out[:])
            dvbc_ps = ps_misc.tile([128, TOK], F32, tag="misc")
            nc.tensor.matmul(out=dvbc_ps[:], lhsT=scale_row[:], rhs=dv_own[:],
                             start=True, stop=True)
            dv_bc = sb.tile([128, TOK], F32)
            nc.vector.tensor_copy(dv_bc[:], dvbc_ps[:])

            # ------- scale + gelu + mm2 + bias + mm3-contribution ----------
            # a-stream first: fully local (hides the ReduceScatter);
            # v-stream after (dv-gated). mm3 accumulates per-stream into one
            # wide PSUM tile.
            o_ps = ps_pe.tile([128, OC, TOK], F32, name="op", tag="pe")                 if not SKIP_MLP else None
            for si, (s, wp, bc, b1off, bmoff) in enumerate((
                ("a", sb_wa, da_bc, 16, 36),
                ("v", sb_wv, dv_bc, 0, 32),
            ) if not SKIP_MLP else ()):
                hsb = sb.tile([128, HC, TOK], BF, name=f"h_{s}")
                sc_sb = sb.tile([128, HC, TOK], BF, name=f"sc_{s}")
                for hc in range(HC):
                    nc.vector.tensor_mul(sc_sb[:, hc, :], z_sb[s][:, hc, :], bc[:])
                for hc in range(HC):
                    nc.scalar.activation(
                        hsb[:, hc, :], sc_sb[:, hc, :],
                        mybir.ActivationFunctionType.Gelu,
                        bias=sb_gf[:, ds(DC * GF_W + b1off + hc, 1)], scale=1.0,
                    )
                hf = sb.tile([128, DC, TOK], BF, name=f"hf_{s}")
                for dc in range(DC):
                    h2 = ps_pe.tile([128, TOK], F32, name="h2", tag="pe")
                    for hc in range(HC):
                        nc.tensor.matmul(
                            out=h2[:],
                            lhsT=wp[:, ds(WP_WM + hc * 512 + dc * 128, 128)],
                            rhs=hsb[:, hc, :],
                            start=(hc == 0), stop=(hc == HC - 1),
                        )
                    nc.vector.tensor_scalar_add(
                        out=hf[:, dc, :], in0=h2[:],
                        scalar1=sb_gf[:, ds(DC * GF_W + bmoff + dc, 1)])
                for oc in range(OC):
                    for dc in range(DC):
                        nc.tensor.matmul(
                            out=o_ps[:, oc, :],
                            lhsT=wp[:, ds(WP_WO + dc * 512 + oc * 128, 128)],
                            rhs=hf[:, dc, :],
                            start=(si == 0 and oc == 0 and dc == 0),
                            stop=(si == 1 and oc == OC - 1 and dc == DC - 1),
                        )

            # ---------------- bias + output ----------------
            out_sb = sb.tile([128, OC, TOK], F32)
            if SKIP_MLP:
                nc.vector.tensor_copy(out_sb[:, 0, :], dv_bc[:])
            for oc in range(OC if not SKIP_MLP else 0):
                nc.vector.tensor_scalar_add(
                    out=out_sb[:, oc, :], in0=o_ps[:, oc, :],
                    scalar1=sb_gf[:, ds(DC * GF_W + 40 + oc, 1)])
            nc.sync.dma_start(out_d.rearrange("o p t -> p o t"), out_sb[:])

    _split_multi_waits(nc)
    return nc


def _chunk(a, nchunk):
    """[nchunk*128, X] row-major -> [128, nchunk*X] per-partition pack."""
    X = a.shape[1]
    return np.ascontiguousarray(
        a.reshape(nchunk, 128, X).transpose(1, 0, 2).reshape(128, nchunk * X))


def make_in_maps(inputs):
    f32 = np.float32
    x_v = np.asarray(inputs["x_v"], f32)
    x_a = np.asarray(inputs["x_a"], f32)
    W1 = {"v": np.asarray(inputs["W1v"], f32), "a": np.asarray(inputs["W1a"], f32)}
    Wm = {"v": np.asarray(inputs["Wmv"], f32), "a": np.asarray(inputs["Wma"], f32)}
    Wout = np.asarray(inputs["Wout"], f32)
    Wo = {"v": Wout[:D], "a": Wout[D:]}
    b1 = {"v": np.asarray(inputs["b1v"], f32), "a": np.asarray(inputs["b1a"], f32)}
    bm = {"v": np.asarray(inputs["bmv"], f32), "a": np.asarray(inputs["bma"], f32)}
    bout = np.asarray(inputs["bout"], f32)

    wpack = {}
    for s in ("v", "a"):
        wpack[s] = np.concatenate(
            [_chunk(W1[s], DC), _chunk(Wm[s], HC), _chunk(Wo[s], DC)], axis=1
        ).astype(ml_dtypes.bfloat16)

    in_maps = []
    for c in range(NCORES):
        b, g = divmod(c, GROUP)
        sl = slice(g * TOK, (g + 1) * TOK)
        xvT = np.ascontiguousarray(x_v[b].T)  # [D, N]
        xaT = np.ascontiguousarray(x_a[b].T)
        # genpack_bf: per dc: [xvT(256) | xvO(64) | xaO(64)]
        gbf = np.zeros((128, DC, GBF_W), f32)
        gbf[:, :, :256] = xvT.reshape(DC, 128, N).transpose(1, 0, 2)
        gbf[:, :, 256:320] = xvT[:, sl].reshape(DC, 128, TOK).transpose(1, 0, 2)
        gbf[:, :, 320:384] = xaT[:, sl].reshape(DC, 128, TOK).transpose(1, 0, 2)
        gf = np.zeros((128, DC, GF_W), f32)
        xac = xaT[:, sl].reshape(DC, 128, TOK).transpose(1, 0, 2)
        gf[:, :, :64] = xac
        gf[:, :, 64:] = -xac
        bias = np.zeros((128, BIAS_W), f32)
        bias[:, 0:16] = b1["v"].reshape(16, 128).T
        bias[:, 16:32] = b1["a"].reshape(16, 128).T
        bias[:, 32:36] = bm["v"].reshape(4, 128).T
        bias[:, 36:40] = bm["a"].reshape(4, 128).T
        bias[:, 40:44] = bout.reshape(4, 128).T
        in_maps.append({
            "g_bf": np.ascontiguousarray(
                gbf.reshape(128, DC * GBF_W)).astype(ml_dtypes.bfloat16),
            "g_f": np.ascontiguousarray(np.concatenate(
                [gf.reshape(128, DC * GF_W), bias], axis=1)),
            "w_v": wpack["v"],
            "w_a": wpack["a"],
        })
    return in_maps


_CACHE = {}
LAST_PERF = {}


def kernel(**inputs) -> np.ndarray:
    if "nc" not in _CACHE:
        _CACHE["nc"] = build_bass()
    nc = _CACHE["nc"]
    in_maps = make_in_maps(inputs)
    trace = bool(int(os.environ.get("KERNEL_TRACE", "0")))
    if trace:
        try:
            import antenv.axon_hooks  # noqa: F401
        except ModuleNotFoundError:
            trace = False  # axon NTFF hook unavailable in this container
    res = run_bass_kernel_spmd(
        nc, in_maps, core_ids=list(range(NCORES)), has_collectives=True,
        trace=trace,
    )
    LAST_PERF["exec_time_ns"] = res.exec_time_ns
    LAST_PERF["trace"] = res.instructions_and_trace
    out = np.zeros((B, N, D), np.float32)
    for c in range(NCORES):
        b, g = divmod(c, GROUP)
        o = res.results[c]["out"]  # [OC, 128, TOK]
        out[b, g * TOK:(g + 1) * TOK] = o.transpose(2, 0, 1).reshape(TOK, D)
    return out


if __name__ == "__main__":
    # static wait-count validation
    import json
    nc = build_bass()
    bir = json.loads(nc.to_json_bytes())
    bad = 0
    for f in bir["functions"]:
        for blk in f["blocks"]:
            for ins in blk["instructions"]:
                si = ins.get("sync_info") or {}
                ow = si.get("on_wait") or []
                if len(ow) > 1:
                    bad += 1
                    print(f"{ins.get('name')} {ins.get('opcode')}: "
                          f"{len(ow)} waits: {[w.get('ant_name') for w in ow]}")
    print(f"validation: {bad} instructions with >1 wait")


# revision 29
# speedup vs baseline: 1.1284x; 1.0004x over previous
"""Trainium2 Bass kernel for nn_DistanceFusionBlock (retrieval_knn).

Sharding (8 NeuronCores, SPMD single NEFF): token-parallel — core c
handles batch b = c // 4, token quarter g = c % 4 (64 tokens) for BOTH
the v- and a-streams. Inputs arrive host-packed per core (transposed,
chunked, bf16) so no on-device transposes are needed.

Distance phase (the N^2*D part), using |x| = 2*relu(x) - x:
  - 256 gen tiles per core: t = relu(x_v[d,:] - x_a[d,j]) over all 256
    i (free dim), d-chunks on partitions, j in the core's own quarter.
    Split DVE tensor_scalar(sub,max0) [4x mode, 127ns] / ACT Relu with
    per-partition bias [398ns] at ACT_EVERY.
  - The PE folds every tile into row j of a [64,256] PSUM "rows" matrix
    via a sliding one-hot-column lhsT (matmul out base-partition must be
    0/32/64, so scattering is done with the weights, accumulating exact
    zeros elsewhere).
  - sum(diff) corrections are analytic from row/col sums of x_v / x_a
    (tiny PE folds): da_raw[j] = 2*rowsum_j - SV + 256*sa_j (local);
    dv partial = 2*colsum - 64*sv + SA, summed across the 4-core group
    by a 1KB ReduceScatter that also hands each core exactly its own
    64 tokens' slice.

MLP phase: features-on-partitions end-to-end; mm1 runs on RAW inputs
interleaved into the PE fold stream (row scaling commutes:
(dv*x) @ W = dv * (x @ W)); the dv/da scale is applied to the mm1
output (dv broadcast across partitions via a K=1 matmul that also
applies the 1/N), then gelu(+per-partition bias) on ACT, mm2, and the
concat-projection as one wide [128,4,64] PSUM accumulation over both
streams. bf16 operands, fp32 accumulation. The a-stream tail is fully
local and hides the ReduceScatter; only the v-stream tail is dv-gated.

Hardware constraint honored throughout: every TPB instruction has ONE
semaphore wait slot (see _split_multi_waits); per-engine absorber ops
retire each DMA-pack semaphore once so hot-loop ops carry at most one.
"""
import os
import sys

sys.path.insert(0, "/opt/trn_rl_repo")

import numpy as np
import ml_dtypes

import concourse.bass as bass
import concourse.mybir as mybir
import concourse.tile as tile
from concourse.bass import ds
from concourse.bass_utils import run_bass_kernel_spmd

B, N, D, H = 2, 256, 512, 2048
NCORES, GROUP, TOK = 8, 4, 64
DC, HC, OC = D // 128, H // 128, D // 128  # 4, 16, 4
BF, F32 = mybir.dt.bfloat16, mybir.dt.float32
ACT_EVERY = 4  # every ACT_EVERY-th gen tile goes to the scalar engine
SKIP_GEN = False
GEN_BUFS = 8
MM1_BASE_V = 72
MM1_BASE_A = 112
SKIP_MLP = False
SKIP_RS = False

# genpack_bf free-dim layout per d-chunk: [xvT(256) | xvO(64) | xaO(64)]
GBF_W = 384
# genpack_f32 layout per d-chunk: [xa_col(64) | -xa_col(64)]
GF_W = 128
# biaspack layout: [b1v(16) | b1a(16) | bmv(4) | bma(4) | bout(4)]
BIAS_W = 44
# weight pack layout (per stream): [W1(4*2048) | Wm(16*512) | Wout_half(4*512)]
WP_W1, WP_WM, WP_WO = 0, 4 * 2048, 4 * 2048 + 16 * 512
WP_W = WP_WO + 4 * 512  # 18432


def _split_multi_waits(nc):
    """Every TPB instruction struct has exactly ONE semaphore-wait slot;
    this snapshot's Tile doesn't split multi-wait instructions (its wait
    optimizer is disabled). Move all-but-one wait of any instruction onto
    injected same-engine NoOps placed immediately before it."""
    import bass_rust
    n = 0
    for fn in nc.m.functions:
        for blk in fn.blocks:
            out = []
            for ins in blk.instructions:
                si = ins.sync_info
                waits = list(si.on_wait) if si is not None and si.on_wait else []
                if len(waits) > 1:
                    for w in waits[:-1]:
                        nop = bass_rust.InstNoOp(
                            name=f"waitsplit-{n}", engine=ins.engine,
                            ins=[], outs=[])
                        nop.sync_info = mybir.SyncInfo(on_wait=[w], on_update=[])
                        out.append(nop)
                        n += 1
                    si.on_wait = [waits[-1]]
                out.append(ins)
            blk.instructions[:] = out
    return n


def build_bass():
    nc = bass.Bass(num_devices=NCORES)
    g_bf = nc.dram_tensor("g_bf", [128, DC * GBF_W], BF, kind="ExternalInput")
    g_f = nc.dram_tensor("g_f", [128, DC * GF_W + BIAS_W], F32, kind="ExternalInput")
    w_v = nc.dram_tensor("w_v", [128, WP_W], BF, kind="ExternalInput")
    w_a = nc.dram_tensor("w_a", [128, WP_W], BF, kind="ExternalInput")
    out_d = nc.dram_tensor("out", [OC, 128, TOK], F32, kind="ExternalOutput")

    with tile.TileContext(nc) as tc:
        with (
            tc.tile_pool(name="inp", bufs=1) as inp,
            tc.tile_pool(name="gen_d", bufs=GEN_BUFS) as genp_d,
            tc.tile_pool(name="diffp", bufs=3) as diffp,
            tc.tile_pool(name="gen_a", bufs=4) as genp_a,
            tc.tile_pool(name="sb", bufs=1) as sb,
            tc.tile_pool(name="ps_acc", bufs=1, space="PSUM") as ps_acc,
            tc.tile_pool(name="ps_misc", bufs=1, space="PSUM") as ps_misc,
            tc.tile_pool(name="ps_pe", bufs=4, space="PSUM") as ps_pe,
            tc.tile_pool(name="ps_dve", bufs=2, space="PSUM") as ps_dve,
            tc.tile_pool(name="dram", bufs=1, space="DRAM") as dram,
        ):
            # ---------------- input DMAs ----------------
            sb_gbf = inp.tile([128, DC * GBF_W], BF)
            sb_gf = inp.tile([128, DC * GF_W + BIAS_W], F32)
            sb_wv = inp.tile([128, WP_W], BF)
            sb_wa = inp.tile([128, WP_W], BF)
            nc.sync.dma_start(sb_gf[:], g_f[:])
            for dc in range(DC):
                nc.sync.dma_start(sb_gbf[:, ds(dc * GBF_W, GBF_W)],
                                  g_bf[:, ds(dc * GBF_W, GBF_W)])
            if not SKIP_MLP:
                nc.sync.dma_start(sb_wv[:, ds(WP_W1, WP_WM)], w_v[:, ds(WP_W1, WP_WM)])
                nc.sync.dma_start(sb_wa[:, ds(WP_W1, WP_WM)], w_a[:, ds(WP_W1, WP_WM)])
                nc.sync.dma_start(sb_wv[:, ds(WP_WM, WP_W - WP_WM)], w_v[:, ds(WP_WM, WP_W - WP_WM)])
                nc.sync.dma_start(sb_wa[:, ds(WP_WM, WP_W - WP_WM)], w_a[:, ds(WP_WM, WP_W - WP_WM)])
            else:
                nc.sync.dma_start(sb_wv[:, 0:2], w_v[:, 0:2])
                nc.sync.dma_start(sb_wa[:, 0:2], w_a[:, 0:2])

            # ---------------- constants ----------------
            zeros = sb.tile([128, 256], BF)
            ones_bf = sb.tile([128, 1], BF)
            ones_f = sb.tile([128, 1], F32)
            c64_bf = sb.tile([128, 1], BF)
            c2_f = sb.tile([128, 1], F32)
            scale_row = sb.tile([1, 128], F32)
            zo = sb.tile([128, 128], BF)  # single ones-column at index TOK
            ident = sb.tile([TOK, TOK], F32)
            nc.vector.memset(zeros[:], 0.0)
            nc.vector.memset(ones_bf[:], 1.0)
            nc.vector.memset(ones_f[:], 1.0)
            nc.vector.memset(c64_bf[:], float(TOK))
            nc.vector.memset(c2_f[:], 2.0)
            nc.vector.memset(scale_row[:], 1.0 / N)
            nc.vector.memset(zo[:], 0.0)
            nc.vector.memset(zo[:, TOK:TOK + 1], 1.0)
            from concourse.masks import make_identity
            make_identity(nc, ident[:])

            # ---------------- per-engine semaphore absorbers ----------------
            # DVE: touch each DMA pack once (1 wait per op, dataflow-safe by
            # priority order).
            dve_scr = sb.tile([1, 2], F32)
            nc.vector.tensor_copy(dve_scr[0:1, 0:1], sb_gf[0:1, 0:1])
            dve_scr2 = sb.tile([1, 2], BF)
            nc.vector.tensor_copy(dve_scr2[0:1, 0:1], sb_gbf[0:1, 0:1])
            # ACT: same, plus warm the gelu/abs table set early.
            act_scr = sb.tile([1, 2], BF)
            nc.scalar.copy(act_scr[0:1, 0:1], sb_gbf[0:1, 0:1])
            act_scr2 = sb.tile([1, 2], F32)
            nc.scalar.copy(act_scr2[0:1, 0:1], sb_gf[0:1, 0:1])
            warm = sb.tile([128, 1], BF)
            nc.scalar.activation(warm[:], zeros[:, 0:1],
                                 mybir.ActivationFunctionType.Gelu)
            # PE: dummy 1-col matmuls absorbing each pack's semaphore.
            scr_ps = ps_misc.tile([1, 1], F32, tag="misc")
            nc.tensor.matmul(out=scr_ps[:], lhsT=ones_bf[:], rhs=ones_bf[:],
                             start=True, stop=True)
            scr_ps2 = ps_misc.tile([1, 1], F32, name="scr2", tag="misc")
            nc.tensor.matmul(out=scr_ps2[:], lhsT=ones_bf[:],
                             rhs=sb_gbf[:, 0:1], start=True, stop=True)

            # ---------------- distance phase ----------------
            # relu trick: |x| = 2*relu(x) - x, and sum(diff) is analytic.
            # Each tile t = relu(x_v[d,:] - x_a[d,j]); folds write row j of
            # rows_ps via a sliding one-hot column lhsT.
            # sv/sa ingredient folds first (their DVE tail overlaps gen)
            sv64_ps = ps_misc.tile([1, 256], F32, tag="misc")
            for dc in range(DC):
                nc.tensor.matmul(out=sv64_ps[:], lhsT=c64_bf[:],
                                 rhs=sb_gbf[:, ds(dc * GBF_W, 256)],
                                 start=(dc == 0), stop=(dc == DC - 1))
            sv64_sb = sb.tile([1, 256], F32)
            nc.vector.tensor_copy(sv64_sb[:], sv64_ps[:])
            sa_ps = ps_misc.tile([1, TOK], F32, tag="misc")
            for dc in range(DC):
                nc.tensor.matmul(out=sa_ps[:], lhsT=ones_bf[:],
                                 rhs=sb_gbf[:, ds(dc * GBF_W + 320, TOK)],
                                 start=(dc == 0), stop=(dc == DC - 1))
            sa_sb = sb.tile([1, TOK], F32)
            nc.vector.tensor_copy(sa_sb[:], sa_ps[:])
            sa_tot = sb.tile([1, 1], F32)
            nc.vector.tensor_reduce(sa_tot[:], sa_sb[:],
                                    axis=mybir.AxisListType.X,
                                    op=mybir.AluOpType.add)
            svq = sb.tile([1, 1], F32)
            nc.vector.tensor_reduce(svq[:], sv64_sb[:],
                                    axis=mybir.AxisListType.X,
                                    op=mybir.AluOpType.add)
            sv_tot = sb.tile([1, 1], F32)
            nc.vector.tensor_scalar(
                out=sv_tot[:], in0=svq[:], scalar1=1.0 / TOK, scalar2=None,
                op0=mybir.AluOpType.mult, op1=mybir.AluOpType.bypass)
            rows_ps = ps_acc.tile([TOK, 256], F32)
            njj = TOK if not SKIP_GEN else 1
            nfold = njj * DC
            k = 0
            for dc in range(DC):
                for j in range(njj):
                    use_act = k % ACT_EVERY == ACT_EVERY - 1
                    t = (genp_a if use_act else genp_d).tile(
                        [128, 256], BF, name="gt")
                    if use_act:
                        nc.scalar.activation(
                            t[:], sb_gbf[:, ds(dc * GBF_W, 256)],
                            mybir.ActivationFunctionType.Relu,
                            bias=sb_gf[:, ds(dc * GF_W + 64 + j, 1)],
                            scale=1.0,
                        )
                    else:
                        nc.vector.tensor_scalar(
                            out=t[:],
                            in0=sb_gbf[:, ds(dc * GBF_W, 256)],
                            scalar1=sb_gf[:, ds(dc * GF_W + j, 1)],
                            scalar2=0.0,
                            op0=mybir.AluOpType.subtract,
                            op1=mybir.AluOpType.max,
                        )
                    nc.tensor.matmul(
                        out=rows_ps[:], lhsT=zo[:, ds(TOK - j, TOK)],
                        rhs=t[:], start=(k == 0), stop=(k == nfold - 1))
                    k += 1
            # rows -> SBUF
            rows_sb = sb.tile([TOK, 256], F32)
            nc.vector.tensor_copy(rows_sb[:], rows_ps[:])

            # -------- dv payload + ReduceScatter dispatch (ASAP) ----------
            # payload[i] = 2*colsum(rows)[i] - 64*sv[i] + SA, fused:
            # colsum folds the 2x into the weights; one STT adds SA and
            # subtracts 64*sv.
            dvr_ps = ps_misc.tile([1, 256], F32, tag="misc")
            nc.tensor.matmul(out=dvr_ps[:], lhsT=c2_f[0:TOK, :],
                             rhs=rows_sb[:], start=True, stop=True)
            dvr_sb = sb.tile([1, 256], F32)
            nc.vector.tensor_copy(dvr_sb[:], dvr_ps[:])
            dvp_sb = sb.tile([1, 256], F32)
            nc.vector.scalar_tensor_tensor(
                out=dvp_sb[:], in0=dvr_sb[:], scalar=sa_tot[:],
                in1=sv64_sb[:], op0=mybir.AluOpType.add,
                op1=mybir.AluOpType.subtract)
            dvp_n = sb.tile([1, 256], F32)
            nc.vector.tensor_scalar(
                out=dvp_n[:], in0=dvp_sb[:], scalar1=1.0 / N, scalar2=None,
                op0=mybir.AluOpType.mult, op1=mybir.AluOpType.bypass)
            rs_in = dram.tile([1, 256], F32)
            rs_out = dram.tile([1, TOK], F32)
            nc.sync.dma_start(rs_in[:], dvp_n[:])
            if not SKIP_RS:
                nc.gpsimd.collective_compute(
                    "ReduceScatter", mybir.AluOpType.add,
                    replica_groups=[[0, 1, 2, 3], [4, 5, 6, 7]],
                    ins=[rs_in.opt()], outs=[rs_out.opt()],
                )
            else:
                nc.sync.dma_start(rs_out[:], rs_in[:, 0:TOK])


            # ---------------- da (local, from rows + analytic corr) -------
            # da_raw[j] = 2*sum_i rows[j,i] - SV + 256*sa[j]
            rowsum = sb.tile([TOK, 1], F32)
            nc.vector.tensor_reduce(rowsum[:], rows_sb[:],
                                    axis=mybir.AxisListType.X,
                                    op=mybir.AluOpType.add)
            rs_t_ps = ps_misc.tile([1, TOK], F32, tag="misc")
            nc.tensor.transpose(rs_t_ps[:], rowsum[:], ident[:])
            rowsum_row = sb.tile([1, TOK], F32)
            nc.vector.tensor_copy(rowsum_row[:], rs_t_ps[:])
            t2_da = sb.tile([1, TOK], F32)
            nc.vector.tensor_scalar(
                out=t2_da[:], in0=sa_sb[:], scalar1=float(N),
                scalar2=sv_tot[:], op0=mybir.AluOpType.mult,
                op1=mybir.AluOpType.subtract)
            da_row = sb.tile([1, TOK], F32)
            nc.vector.scalar_tensor_tensor(
                out=da_row[:], in0=rowsum_row[:], scalar=2.0, in1=t2_da[:],
                op0=mybir.AluOpType.mult, op1=mybir.AluOpType.add)
            dabc_ps = ps_misc.tile([128, TOK], F32, tag="misc")
            nc.tensor.matmul(out=dabc_ps[:], lhsT=scale_row[:], rhs=da_row[:],
                             start=True, stop=True)
            da_bc = sb.tile([128, TOK], F32)
            nc.vector.tensor_copy(da_bc[:], dabc_ps[:])

            # ---------------- mm1 raw (both streams; overlaps the RS) -------
            z_sb = {}
            for s, wp, xoff in ((("v", sb_wv, 256), ("a", sb_wa, 320))
                                if not SKIP_MLP else ()):
                z_sb[s] = sb.tile([128, HC, TOK], BF, name=f"z_{s}")
                for grp in range(HC // 4):
                    zp = ps_pe.tile([128, 4, TOK], F32, name="zp", tag="pe")
                    for hcm in range(4):
                        hc = grp * 4 + hcm
                        for dcw in range(DC):
                            nc.tensor.matmul(
                                out=zp[:, hcm, :],
                                lhsT=wp[:, ds(WP_W1 + dcw * 2048 + hc * 128, 128)],
                                rhs=sb_gbf[:, ds(dcw * GBF_W + xoff, TOK)],
                                start=(dcw == 0), stop=(dcw == DC - 1),
                            )
                    nc.vector.tensor_copy(z_sb[s][:, ds(grp * 4, 4), :], zp[:])

            # ---------------- dv readback (partition-broadcast DMA) --------
            dv_bc = sb.tile([128, TOK], F32)
            nc.sync.dma_start(dv_bc[:],
                              rs_out[0:1, :].partition_broadcast(128))

            # ------- scale + gelu + mm2 + bias + mm3-contribution ----------
            # a-stream first: fully local (hides the ReduceScatter);
            # v-stream after (dv-gated). mm3 accumulates per-stream into one
            # wide PSUM tile.
            o_ps = ps_pe.tile([128, OC, TOK], F32, name="op", tag="pe")                 if not SKIP_MLP else None
            for si, (s, wp, bc, b1off, bmoff) in enumerate((
                ("a", sb_wa, da_bc, 16, 36),
                ("v", sb_wv, dv_bc, 0, 32),
            ) if not SKIP_MLP else ()):
                hsb = sb.tile([128, HC, TOK], BF, name=f"h_{s}")
                sc_sb = sb.tile([128, HC, TOK], BF, name=f"sc_{s}")
                for hc in range(HC):
                    nc.vector.tensor_mul(sc_sb[:, hc, :], z_sb[s][:, hc, :], bc[:])
                for hc in range(HC):
                    nc.scalar.activation(
                        hsb[:, hc, :], sc_sb[:, hc, :],
                        mybir.ActivationFunctionType.Gelu,
                        bias=sb_gf[:, ds(DC * GF_W + b1off + hc, 1)], scale=1.0,
                    )
                hf = sb.tile([128, DC, TOK], BF, name=f"hf_{s}")
                for dc in range(DC):
                    h2 = ps_pe.tile([128, TOK], F32, name="h2", tag="pe")
                    for hc in range(HC):
                        nc.tensor.matmul(
                            out=h2[:],
                            lhsT=wp[:, ds(WP_WM + hc * 512 + dc * 128, 128)],
                            rhs=hsb[:, hc, :],
                            start=(hc == 0), stop=(hc == HC - 1),
                        )
                    nc.vector.tensor_scalar_add(
                        out=hf[:, dc, :], in0=h2[:],
                        scalar1=sb_gf[:, ds(DC * GF_W + bmoff + dc, 1)])
                for oc in range(OC):
                    for dc in range(DC):
                        nc.tensor.matmul(
                            out=o_ps[:, oc, :],
                            lhsT=wp[:, ds(WP_WO + dc * 512 + oc * 128, 128)],
                            rhs=hf[:, dc, :],
                            start=(si == 0 and oc == 0 and dc == 0),
                            stop=(si == 1 and oc == OC - 1 and dc == DC - 1),
                        )

            # ---------------- bias + output ----------------
            out_sb = sb.tile([128, OC, TOK], F32)
            if SKIP_MLP:
                nc.vector.tensor_copy(out_sb[:, 0, :], dv_bc[:])
            for oc in range(OC if not SKIP_MLP else 0):
                nc.vector.tensor_scalar_add(
                    out=out_sb[:, oc, :], in0=o_ps[:, oc, :],
                    scalar1=sb_gf[:, ds(DC * GF_W + 40 + oc, 1)])
            nc.sync.dma_start(out_d.rearrange("o p t -> p o t"), out_sb[:])

    _split_multi_waits(nc)
    return nc


def _chunk(a, nchunk):
    """[nchunk*128, X] row-major -> [128, nchunk*X] per-partition pack."""
    X = a.shape[1]
    return np.ascontiguousarray(
        a.reshape(nchunk, 128, X).transpose(1, 0, 2).reshape(128, nchunk * X))


def make_in_maps(inputs):
    f32 = np.float32
    x_v = np.asarray(inputs["x_v"], f32)
    x_a = np.asarray(inputs["x_a"], f32)
    W1 = {"v": np.asarray(inputs["W1v"], f32), "a": np.asarray(inputs["W1a"], f32)}
    Wm = {"v": np.asarray(inputs["Wmv"], f32), "a": np.asarray(inputs["Wma"], f32)}
    Wout = np.asarray(inputs["Wout"], f32)
    Wo = {"v": Wout[:D], "a": Wout[D:]}
    b1 = {"v": np.asarray(inputs["b1v"], f32), "a": np.asarray(inputs["b1a"], f32)}
    bm = {"v": np.asarray(inputs["bmv"], f32), "a": np.asarray(inputs["bma"], f32)}
    bout = np.asarray(inputs["bout"], f32)

    wpack = {}
    for s in ("v", "a"):
        wpack[s] = np.concatenate(
            [_chunk(W1[s], DC), _chunk(Wm[s], HC), _chunk(Wo[s], DC)], axis=1
        ).astype(ml_dtypes.bfloat16)

    in_maps = []
    for c in range(NCORES):
        b, g = divmod(c, GROUP)
        sl = slice(g * TOK, (g + 1) * TOK)
        xvT = np.ascontiguousarray(x_v[b].T)  # [D, N]
        xaT = np.ascontiguousarray(x_a[b].T)
        # genpack_bf: per dc: [xvT(256) | xvO(64) | xaO(64)]
        gbf = np.zeros((128, DC, GBF_W), f32)
        gbf[:, :, :256] = xvT.reshape(DC, 128, N).transpose(1, 0, 2)
        gbf[:, :, 256:320] = xvT[:, sl].reshape(DC, 128, TOK).transpose(1, 0, 2)
        gbf[:, :, 320:384] = xaT[:, sl].reshape(DC, 128, TOK).transpose(1, 0, 2)
        gf = np.zeros((128, DC, GF_W), f32)
        xac = xaT[:, sl].reshape(DC, 128, TOK).transpose(1, 0, 2)
        gf[:, :, :64] = xac
        gf[:, :, 64:] = -xac
        bias = np.zeros((128, BIAS_W), f32)
        bias[:, 0:16] = b1["v"].reshape(16, 128).T
        bias[:, 16:32] = b1["a"].reshape(16, 128).T
        bias[:, 32:36] = bm["v"].reshape(4, 128).T
        bias[:, 36:40] = bm["a"].reshape(4, 128).T
        bias[:, 40:44] = bout.reshape(4, 128).T
        in_maps.append({
            "g_bf": np.ascontiguousarray(
                gbf.reshape(128, DC * GBF_W)).astype(ml_dtypes.bfloat16),
            "g_f": np.ascontiguousarray(np.concatenate(
                [gf.reshape(128, DC * GF_W), bias], axis=1)),
            "w_v": wpack["v"],
            "w_a": wpack["a"],
        })
    return in_maps


_CACHE = {}
LAST_PERF = {}


def kernel(**inputs) -> np.ndarray:
    if "nc" not in _CACHE:
        _CACHE["nc"] = build_bass()
    nc = _CACHE["nc"]
    in_maps = make_in_maps(inputs)
    trace = bool(int(os.environ.get("KERNEL_TRACE", "0")))
    if trace:
        try:
            import antenv.axon_hooks  # noqa: F401
        except ModuleNotFoundError:
            trace = False  # axon NTFF hook unavailable in this container
    res = run_bass_kernel_spmd(
        nc, in_maps, core_ids=list(range(NCORES)), has_collectives=True,
        trace=trace,
    )
    LAST_PERF["exec_time_ns"] = res.exec_time_ns
    LAST_PERF["trace"] = res.instructions_and_trace
    out = np.zeros((B, N, D), np.float32)
    for c in range(NCORES):
        b, g = divmod(c, GROUP)
        o = res.results[c]["out"]  # [OC, 128, TOK]
        out[b, g * TOK:(g + 1) * TOK] = o.transpose(2, 0, 1).reshape(TOK, D)
    return out


if __name__ == "__main__":
    # static wait-count validation
    import json
    nc = build_bass()
    bir = json.loads(nc.to_json_bytes())
    bad = 0
    for f in bir["functions"]:
        for blk in f["blocks"]:
            for ins in blk["instructions"]:
                si = ins.get("sync_info") or {}
                ow = si.get("on_wait") or []
                if len(ow) > 1:
                    bad += 1
                    print(f"{ins.get('name')} {ins.get('opcode')}: "
                          f"{len(ow)} waits: {[w.get('ant_name') for w in ow]}")
    print(f"validation: {bad} instructions with >1 wait")


# revision 33
# speedup vs baseline: 1.1459x; 1.0155x over previous
"""Trainium2 Bass kernel for nn_DistanceFusionBlock (retrieval_knn).

Sharding (8 NeuronCores, SPMD single NEFF): token-parallel — core c
handles batch b = c // 4, token quarter g = c % 4 (64 tokens) for BOTH
the v- and a-streams. Inputs arrive host-packed per core (transposed,
chunked, bf16) so no on-device transposes are needed.

Distance phase (the N^2*D part), using |x| = 2*relu(x) - x:
  - 256 gen tiles per core: t = relu(x_v[d,:] - x_a[d,j]) over all 256
    i (free dim), d-chunks on partitions, j in the core's own quarter.
    Split DVE tensor_scalar(sub,max0) [4x mode, 127ns] / ACT Relu with
    per-partition bias [398ns] at ACT_EVERY.
  - The PE folds every tile into row j of a [64,256] PSUM "rows" matrix
    via a sliding one-hot-column lhsT (matmul out base-partition must be
    0/32/64, so scattering is done with the weights, accumulating exact
    zeros elsewhere).
  - sum(diff) corrections are analytic from row/col sums of x_v / x_a
    (tiny PE folds): da_raw[j] = 2*rowsum_j - SV + 256*sa_j (local);
    dv partial = 2*colsum - 64*sv + SA, summed across the 4-core group
    by a 1KB ReduceScatter that also hands each core exactly its own
    64 tokens' slice.

MLP phase: features-on-partitions end-to-end; mm1 runs on RAW inputs
interleaved into the PE fold stream (row scaling commutes:
(dv*x) @ W = dv * (x @ W)); the dv/da scale is applied to the mm1
output (dv broadcast across partitions via a K=1 matmul that also
applies the 1/N), then gelu(+per-partition bias) on ACT, mm2, and the
concat-projection as one wide [128,4,64] PSUM accumulation over both
streams. bf16 operands, fp32 accumulation. The a-stream tail is fully
local and hides the ReduceScatter; only the v-stream tail is dv-gated.

Hardware constraint honored throughout: every TPB instruction has ONE
semaphore wait slot (see _split_multi_waits); per-engine absorber ops
retire each DMA-pack semaphore once so hot-loop ops carry at most one.
"""
import os
import sys

sys.path.insert(0, "/opt/trn_rl_repo")

import numpy as np
import ml_dtypes

import concourse.bass as bass
import concourse.mybir as mybir
import concourse.tile as tile
from concourse.bass import ds
from concourse.bass_utils import run_bass_kernel_spmd

B, N, D, H = 2, 256, 512, 2048
NCORES, GROUP, TOK = 8, 4, 64
DC, HC, OC = D // 128, H // 128, D // 128  # 4, 16, 4
BF, F32 = mybir.dt.bfloat16, mybir.dt.float32
ACT_EVERY = 4  # every ACT_EVERY-th gen tile goes to the scalar engine
SKIP_GEN = False
GEN_BUFS = 8
MM1_BASE_V = 72
MM1_BASE_A = 112
SKIP_MLP = False
SKIP_RS = False

# genpack_bf free-dim layout per d-chunk: [xvT(256) | xvO(64) | xaO(64)]
GBF_W = 384
# genpack_f32 layout per d-chunk: [xa_col(64) | -xa_col(64)]
GF_W = 128
# biaspack layout: [b1v(16) | b1a(16) | bmv(4) | bma(4) | bout(4)]
BIAS_W = 44
# weight pack layout (per stream): [W1(4*2048) | Wm(16*512) | Wout_half(4*512)]
WP_W1, WP_WM, WP_WO = 0, 4 * 2048, 4 * 2048 + 16 * 512
WP_W = WP_WO + 4 * 512  # 18432


def _split_multi_waits(nc):
    """Every TPB instruction struct has exactly ONE semaphore-wait slot;
    this snapshot's Tile doesn't split multi-wait instructions (its wait
    optimizer is disabled). Move all-but-one wait of any instruction onto
    injected same-engine NoOps placed immediately before it."""
    import bass_rust
    n = 0
    for fn in nc.m.functions:
        for blk in fn.blocks:
            out = []
            for ins in blk.instructions:
                si = ins.sync_info
                waits = list(si.on_wait) if si is not None and si.on_wait else []
                if len(waits) > 1:
                    for w in waits[:-1]:
                        nop = bass_rust.InstNoOp(
                            name=f"waitsplit-{n}", engine=ins.engine,
                            ins=[], outs=[])
                        nop.sync_info = mybir.SyncInfo(on_wait=[w], on_update=[])
                        out.append(nop)
                        n += 1
                    si.on_wait = [waits[-1]]
                out.append(ins)
            blk.instructions[:] = out
    return n


def build_bass():
    nc = bass.Bass(num_devices=NCORES)
    g_bf = nc.dram_tensor("g_bf", [128, DC * GBF_W], BF, kind="ExternalInput")
    g_f = nc.dram_tensor("g_f", [128, DC * GF_W + BIAS_W], F32, kind="ExternalInput")
    w_v = nc.dram_tensor("w_v", [128, WP_W], BF, kind="ExternalInput")
    w_a = nc.dram_tensor("w_a", [128, WP_W], BF, kind="ExternalInput")
    out_d = nc.dram_tensor("out", [OC, 128, TOK], F32, kind="ExternalOutput")

    with tile.TileContext(nc) as tc:
        with (
            tc.tile_pool(name="inp", bufs=1) as inp,
            tc.tile_pool(name="gen_d", bufs=GEN_BUFS) as genp_d,
            tc.tile_pool(name="diffp", bufs=3) as diffp,
            tc.tile_pool(name="gen_a", bufs=4) as genp_a,
            tc.tile_pool(name="sb", bufs=1) as sb,
            tc.tile_pool(name="ps_acc", bufs=1, space="PSUM") as ps_acc,
            tc.tile_pool(name="ps_misc", bufs=1, space="PSUM") as ps_misc,
            tc.tile_pool(name="ps_pe", bufs=4, space="PSUM") as ps_pe,
            tc.tile_pool(name="ps_dve", bufs=2, space="PSUM") as ps_dve,
            tc.tile_pool(name="dram", bufs=1, space="DRAM") as dram,
        ):
            # ---------------- input DMAs ----------------
            sb_gbf = inp.tile([128, DC * GBF_W], BF)
            sb_gf = inp.tile([128, DC * GF_W + BIAS_W], F32)
            sb_wv = inp.tile([128, WP_W], BF)
            sb_wa = inp.tile([128, WP_W], BF)
            nc.sync.dma_start(sb_gf[:], g_f[:])
            for dc in range(DC):
                nc.sync.dma_start(sb_gbf[:, ds(dc * GBF_W, GBF_W)],
                                  g_bf[:, ds(dc * GBF_W, GBF_W)])
            if not SKIP_MLP:
                nc.sync.dma_start(sb_wv[:, ds(WP_W1, WP_WM)], w_v[:, ds(WP_W1, WP_WM)])
                nc.sync.dma_start(sb_wa[:, ds(WP_W1, WP_WM)], w_a[:, ds(WP_W1, WP_WM)])
                nc.sync.dma_start(sb_wv[:, ds(WP_WM, WP_W - WP_WM)], w_v[:, ds(WP_WM, WP_W - WP_WM)])
                nc.sync.dma_start(sb_wa[:, ds(WP_WM, WP_W - WP_WM)], w_a[:, ds(WP_WM, WP_W - WP_WM)])
            else:
                nc.sync.dma_start(sb_wv[:, 0:2], w_v[:, 0:2])
                nc.sync.dma_start(sb_wa[:, 0:2], w_a[:, 0:2])

            # ---------------- constants ----------------
            zeros = sb.tile([128, 256], BF)
            ones_bf = sb.tile([128, 1], BF)
            ones_f = sb.tile([128, 1], F32)
            c64_bf = sb.tile([128, 1], BF)
            c2_f = sb.tile([128, 1], F32)
            scale_row = sb.tile([1, 128], F32)
            zo = sb.tile([128, 128], BF)  # single ones-column at index TOK
            ident = sb.tile([TOK, TOK], F32)
            nc.vector.memset(zeros[:], 0.0)
            nc.vector.memset(ones_bf[:], 1.0)
            nc.vector.memset(ones_f[:], 1.0)
            nc.vector.memset(c64_bf[:], float(TOK) / N)
            cinv_bf = sb.tile([128, 1], BF)
            nc.vector.memset(cinv_bf[:], 1.0 / N)
            nc.vector.memset(c2_f[:], 2.0 / N)
            nc.vector.memset(scale_row[:], 1.0 / N)
            nc.vector.memset(zo[:], 0.0)
            nc.vector.memset(zo[:, TOK:TOK + 1], 1.0)
            from concourse.masks import make_identity
            make_identity(nc, ident[:])

            # ---------------- per-engine semaphore absorbers ----------------
            # DVE: touch each DMA pack once (1 wait per op, dataflow-safe by
            # priority order).
            dve_scr = sb.tile([1, 2], F32)
            nc.vector.tensor_copy(dve_scr[0:1, 0:1], sb_gf[0:1, 0:1])
            dve_scr2 = sb.tile([1, 2], BF)
            nc.vector.tensor_copy(dve_scr2[0:1, 0:1], sb_gbf[0:1, 0:1])
            # ACT: same, plus warm the gelu/abs table set early.
            act_scr = sb.tile([1, 2], BF)
            nc.scalar.copy(act_scr[0:1, 0:1], sb_gbf[0:1, 0:1])
            act_scr2 = sb.tile([1, 2], F32)
            nc.scalar.copy(act_scr2[0:1, 0:1], sb_gf[0:1, 0:1])
            warm = sb.tile([128, 1], BF)
            nc.scalar.activation(warm[:], zeros[:, 0:1],
                                 mybir.ActivationFunctionType.Gelu)
            # PE: dummy 1-col matmuls absorbing each pack's semaphore.
            scr_ps = ps_misc.tile([1, 1], F32, tag="misc")
            nc.tensor.matmul(out=scr_ps[:], lhsT=ones_bf[:], rhs=ones_bf[:],
                             start=True, stop=True)
            scr_ps2 = ps_misc.tile([1, 1], F32, name="scr2", tag="misc")
            nc.tensor.matmul(out=scr_ps2[:], lhsT=ones_bf[:],
                             rhs=sb_gbf[:, 0:1], start=True, stop=True)

            # ---------------- distance phase ----------------
            # relu trick: |x| = 2*relu(x) - x, and sum(diff) is analytic.
            # Each tile t = relu(x_v[d,:] - x_a[d,j]); folds write row j of
            # rows_ps via a sliding one-hot column lhsT.
            # sv/sa ingredient folds first (their DVE tail overlaps gen)
            sv64_ps = ps_misc.tile([1, 256], F32, tag="misc")
            for dc in range(DC):
                nc.tensor.matmul(out=sv64_ps[:], lhsT=c64_bf[:],
                                 rhs=sb_gbf[:, ds(dc * GBF_W, 256)],
                                 start=(dc == 0), stop=(dc == DC - 1))
            sv64_sb = sb.tile([1, 256], F32)
            nc.vector.tensor_copy(sv64_sb[:], sv64_ps[:])
            sa_ps = ps_misc.tile([1, TOK], F32, tag="misc")
            for dc in range(DC):
                nc.tensor.matmul(out=sa_ps[:], lhsT=cinv_bf[:],
                                 rhs=sb_gbf[:, ds(dc * GBF_W + 320, TOK)],
                                 start=(dc == 0), stop=(dc == DC - 1))
            sa_sb = sb.tile([1, TOK], F32)
            nc.vector.tensor_copy(sa_sb[:], sa_ps[:])
            sa_tot = sb.tile([1, 1], F32)
            nc.vector.tensor_reduce(sa_tot[:], sa_sb[:],
                                    axis=mybir.AxisListType.X,
                                    op=mybir.AluOpType.add)
            svq = sb.tile([1, 1], F32)
            nc.vector.tensor_reduce(svq[:], sv64_sb[:],
                                    axis=mybir.AxisListType.X,
                                    op=mybir.AluOpType.add)
            sv_tot = sb.tile([1, 1], F32)
            nc.vector.tensor_scalar(
                out=sv_tot[:], in0=svq[:], scalar1=float(N) / TOK,
                scalar2=None,
                op0=mybir.AluOpType.mult, op1=mybir.AluOpType.bypass)
            rows_ps = ps_acc.tile([TOK, 256], F32)
            njj = TOK if not SKIP_GEN else 1
            nfold = njj * DC
            k = 0
            for dc in range(DC):
                for j in range(njj):
                    use_act = k % ACT_EVERY == ACT_EVERY - 1
                    t = (genp_a if use_act else genp_d).tile(
                        [128, 256], BF, name="gt")
                    if use_act:
                        nc.scalar.activation(
                            t[:], sb_gbf[:, ds(dc * GBF_W, 256)],
                            mybir.ActivationFunctionType.Relu,
                            bias=sb_gf[:, ds(dc * GF_W + 64 + j, 1)],
                            scale=1.0,
                        )
                    else:
                        nc.vector.tensor_scalar(
                            out=t[:],
                            in0=sb_gbf[:, ds(dc * GBF_W, 256)],
                            scalar1=sb_gf[:, ds(dc * GF_W + j, 1)],
                            scalar2=0.0,
                            op0=mybir.AluOpType.subtract,
                            op1=mybir.AluOpType.max,
                        )
                    nc.tensor.matmul(
                        out=rows_ps[:], lhsT=zo[:, ds(TOK - j, TOK)],
                        rhs=t[:], start=(k == 0), stop=(k == nfold - 1))
                    k += 1
            # rows -> SBUF
            rows_sb = sb.tile([TOK, 256], F32)
            nc.vector.tensor_copy(rows_sb[:], rows_ps[:])

            # -------- dv payload + ReduceScatter dispatch (ASAP) ----------
            # payload[i] = 2*colsum(rows)[i] - 64*sv[i] + SA, fused:
            # colsum folds the 2x into the weights; one STT adds SA and
            # subtracts 64*sv.
            dvr_ps = ps_misc.tile([1, 256], F32, tag="misc")
            nc.tensor.matmul(out=dvr_ps[:], lhsT=c2_f[0:TOK, :],
                             rhs=rows_sb[:], start=True, stop=True)
            dvp_sb = sb.tile([1, 256], F32)
            nc.vector.scalar_tensor_tensor(
                out=dvp_sb[:], in0=dvr_ps[:], scalar=sa_tot[:],
                in1=sv64_sb[:], op0=mybir.AluOpType.add,
                op1=mybir.AluOpType.subtract)
            rs_in = dram.tile([1, 256], F32)
            rs_out = dram.tile([1, TOK], F32)
            nc.sync.dma_start(rs_in[:], dvp_sb[:])
            if not SKIP_RS:
                nc.gpsimd.collective_compute(
                    "ReduceScatter", mybir.AluOpType.add,
                    replica_groups=[[0, 1, 2, 3], [4, 5, 6, 7]],
                    ins=[rs_in.opt()], outs=[rs_out.opt()],
                )
            else:
                nc.sync.dma_start(rs_out[:], rs_in[:, 0:TOK])


            # ---------------- da (local, from rows + analytic corr) -------
            # da_raw[j] = 2*sum_i rows[j,i] - SV + 256*sa[j]
            rowsum = sb.tile([TOK, 1], F32)
            nc.vector.tensor_reduce(rowsum[:], rows_sb[:],
                                    axis=mybir.AxisListType.X,
                                    op=mybir.AluOpType.add)
            rs_t_ps = ps_misc.tile([1, TOK], F32, tag="misc")
            nc.tensor.transpose(rs_t_ps[:], rowsum[:], ident[:])
            rowsum_row = sb.tile([1, TOK], F32)
            nc.vector.tensor_copy(rowsum_row[:], rs_t_ps[:])
            t2_da = sb.tile([1, TOK], F32)
            nc.vector.tensor_scalar(
                out=t2_da[:], in0=sa_sb[:], scalar1=float(N) * float(N),
                scalar2=sv_tot[:], op0=mybir.AluOpType.mult,
                op1=mybir.AluOpType.subtract)
            da_row = sb.tile([1, TOK], F32)
            nc.vector.scalar_tensor_tensor(
                out=da_row[:], in0=rowsum_row[:], scalar=2.0, in1=t2_da[:],
                op0=mybir.AluOpType.mult, op1=mybir.AluOpType.add)
            dabc_ps = ps_misc.tile([128, TOK], F32, tag="misc")
            nc.tensor.matmul(out=dabc_ps[:], lhsT=scale_row[:], rhs=da_row[:],
                             start=True, stop=True)
            da_bc = sb.tile([128, TOK], F32)
            nc.vector.tensor_copy(da_bc[:], dabc_ps[:])

            # ---------------- mm1 raw (both streams; overlaps the RS) -------
            z_sb = {}
            for s, wp, xoff in ((("v", sb_wv, 256), ("a", sb_wa, 320))
                                if not SKIP_MLP else ()):
                z_sb[s] = sb.tile([128, HC, TOK], BF, name=f"z_{s}")
                for grp in range(HC // 4):
                    zp = ps_pe.tile([128, 4, TOK], F32, name="zp", tag="pe")
                    for hcm in range(4):
                        hc = grp * 4 + hcm
                        for dcw in range(DC):
                            nc.tensor.matmul(
                                out=zp[:, hcm, :],
                                lhsT=wp[:, ds(WP_W1 + dcw * 2048 + hc * 128, 128)],
                                rhs=sb_gbf[:, ds(dcw * GBF_W + xoff, TOK)],
                                start=(dcw == 0), stop=(dcw == DC - 1),
                            )
                    nc.vector.tensor_copy(z_sb[s][:, ds(grp * 4, 4), :], zp[:])

            # ---------------- dv readback (partition-broadcast DMA) --------
            dv_bc = sb.tile([128, TOK], F32)
            nc.sync.dma_start(dv_bc[:],
                              rs_out[0:1, :].partition_broadcast(128))

            # ------- scale + gelu + mm2 + bias + mm3-contribution ----------
            # a-stream first: fully local (hides the ReduceScatter);
            # v-stream after (dv-gated). mm3 accumulates per-stream into one
            # wide PSUM tile.
            o_ps = ps_pe.tile([128, OC, TOK], F32, name="op", tag="pe")                 if not SKIP_MLP else None
            for si, (s, wp, bc, b1off, bmoff) in enumerate((
                ("a", sb_wa, da_bc, 16, 36),
                ("v", sb_wv, dv_bc, 0, 32),
            ) if not SKIP_MLP else ()):
                hsb = sb.tile([128, HC, TOK], BF, name=f"h_{s}")
                sc_sb = sb.tile([128, HC, TOK], BF, name=f"sc_{s}")
                for hc in range(HC):
                    nc.vector.tensor_mul(sc_sb[:, hc, :], z_sb[s][:, hc, :], bc[:])
                for hc in range(HC):
                    nc.scalar.activation(
                        hsb[:, hc, :], sc_sb[:, hc, :],
                        mybir.ActivationFunctionType.Gelu,
                        bias=sb_gf[:, ds(DC * GF_W + b1off + hc, 1)], scale=1.0,
                    )
                hf = sb.tile([128, DC, TOK], BF, name=f"hf_{s}")
                for dc in range(DC):
                    h2 = ps_pe.tile([128, TOK], F32, name="h2", tag="pe")
                    for hc in range(HC):
                        nc.tensor.matmul(
                            out=h2[:],
                            lhsT=wp[:, ds(WP_WM + hc * 512 + dc * 128, 128)],
                            rhs=hsb[:, hc, :],
                            start=(hc == 0), stop=(hc == HC - 1),
                        )
                    nc.vector.tensor_scalar_add(
                        out=hf[:, dc, :], in0=h2[:],
                        scalar1=sb_gf[:, ds(DC * GF_W + bmoff + dc, 1)])
                for oc in range(OC):
                    for dc in range(DC):
                        nc.tensor.matmul(
                            out=o_ps[:, oc, :],
                            lhsT=wp[:, ds(WP_WO + dc * 512 + oc * 128, 128)],
                            rhs=hf[:, dc, :],
                            start=(si == 0 and oc == 0 and dc == 0),
                            stop=(si == 1 and oc == OC - 1 and dc == DC - 1),
                        )

            # ---------------- bias + output ----------------
            out_sb = sb.tile([128, OC, TOK], F32)
            if SKIP_MLP:
                nc.vector.tensor_copy(out_sb[:, 0, :], dv_bc[:])
            for oc in range(OC if not SKIP_MLP else 0):
                nc.vector.tensor_scalar_add(
                    out=out_sb[:, oc, :], in0=o_ps[:, oc, :],
                    scalar1=sb_gf[:, ds(DC * GF_W + 40 + oc, 1)])
            nc.sync.dma_start(out_d.rearrange("o p t -> p o t"), out_sb[:])

    _split_multi_waits(nc)
    return nc


def _chunk(a, nchunk):
    """[nchunk*128, X] row-major -> [128, nchunk*X] per-partition pack."""
    X = a.shape[1]
    return np.ascontiguousarray(
        a.reshape(nchunk, 128, X).transpose(1, 0, 2).reshape(128, nchunk * X))


def make_in_maps(inputs):
    f32 = np.float32
    x_v = np.asarray(inputs["x_v"], f32)
    x_a = np.asarray(inputs["x_a"], f32)
    W1 = {"v": np.asarray(inputs["W1v"], f32), "a": np.asarray(inputs["W1a"], f32)}
    Wm = {"v": np.asarray(inputs["Wmv"], f32), "a": np.asarray(inputs["Wma"], f32)}
    Wout = np.asarray(inputs["Wout"], f32)
    Wo = {"v": Wout[:D], "a": Wout[D:]}
    b1 = {"v": np.asarray(inputs["b1v"], f32), "a": np.asarray(inputs["b1a"], f32)}
    bm = {"v": np.asarray(inputs["bmv"], f32), "a": np.asarray(inputs["bma"], f32)}
    bout = np.asarray(inputs["bout"], f32)

    wpack = {}
    for s in ("v", "a"):
        wpack[s] = np.concatenate(
            [_chunk(W1[s], DC), _chunk(Wm[s], HC), _chunk(Wo[s], DC)], axis=1
        ).astype(ml_dtypes.bfloat16)

    in_maps = []
    for c in range(NCORES):
        b, g = divmod(c, GROUP)
        sl = slice(g * TOK, (g + 1) * TOK)
        xvT = np.ascontiguousarray(x_v[b].T)  # [D, N]
        xaT = np.ascontiguousarray(x_a[b].T)
        # genpack_bf: per dc: [xvT(256) | xvO(64) | xaO(64)]
        gbf = np.zeros((128, DC, GBF_W), f32)
        gbf[:, :, :256] = xvT.reshape(DC, 128, N).transpose(1, 0, 2)
        gbf[:, :, 256:320] = xvT[:, sl].reshape(DC, 128, TOK).transpose(1, 0, 2)
        gbf[:, :, 320:384] = xaT[:, sl].reshape(DC, 128, TOK).transpose(1, 0, 2)
        gf = np.zeros((128, DC, GF_W), f32)
        xac = xaT[:, sl].reshape(DC, 128, TOK).transpose(1, 0, 2)
        gf[:, :, :64] = xac
        gf[:, :, 64:] = -xac
        bias = np.zeros((128, BIAS_W), f32)
        bias[:, 0:16] = b1["v"].reshape(16, 128).T
        bias[:, 16:32] = b1["a"].reshape(16, 128).T
        bias[:, 32:36] = bm["v"].reshape(4, 128).T
        bias[:, 36:40] = bm["a"].reshape(4, 128).T
        bias[:, 40:44] = bout.reshape(4, 128).T
        in_maps.append({
            "g_bf": np.ascontiguousarray(
                gbf.reshape(128, DC * GBF_W)).astype(ml_dtypes.bfloat16),
            "g_f": np.ascontiguousarray(np.concatenate(
                [gf.reshape(128, DC * GF_W), bias], axis=1)),
            "w_v": wpack["v"],
            "w_a": wpack["a"],
        })
    return in_maps


_CACHE = {}
LAST_PERF = {}


def kernel(**inputs) -> np.ndarray:
    if "nc" not in _CACHE:
        _CACHE["nc"] = build_bass()
    nc = _CACHE["nc"]
    in_maps = make_in_maps(inputs)
    trace = bool(int(os.environ.get("KERNEL_TRACE", "0")))
    if trace:
        try:
            import antenv.axon_hooks  # noqa: F401
        except ModuleNotFoundError:
            trace = False  # axon NTFF hook unavailable in this container
    res = run_bass_kernel_spmd(
        nc, in_maps, core_ids=list(range(NCORES)), has_collectives=True,
        trace=trace,
    )
    LAST_PERF["exec_time_ns"] = res.exec_time_ns
    LAST_PERF["trace"] = res.instructions_and_trace
    out = np.zeros((B, N, D), np.float32)
    for c in range(NCORES):
        b, g = divmod(c, GROUP)
        o = res.results[c]["out"]  # [OC, 128, TOK]
        out[b, g * TOK:(g + 1) * TOK] = o.transpose(2, 0, 1).reshape(TOK, D)
    return out


if __name__ == "__main__":
    # static wait-count validation
    import json
    nc = build_bass()
    bir = json.loads(nc.to_json_bytes())
    bad = 0
    for f in bir["functions"]:
        for blk in f["blocks"]:
            for ins in blk["instructions"]:
                si = ins.get("sync_info") or {}
                ow = si.get("on_wait") or []
                if len(ow) > 1:
                    bad += 1
                    print(f"{ins.get('name')} {ins.get('opcode')}: "
                          f"{len(ow)} waits: {[w.get('ant_name') for w in ow]}")
    print(f"validation: {bad} instructions with >1 wait")
